# revision 31
# baseline (speedup 1.0000x reference)
"""Multi-head attention on 8 Trainium2 NeuronCores — fp8 DoubleRow version.

Sharding: batch (2) x query-row-block (4) -> 8 cores; each core computes full
attention for its 512 query rows of one batch (K/V projected for all keys).

Techniques vs the fp32r baseline:
  - QKV projections run as fp8e4 DoubleRow matmuls on a host-side double-fp8
    decomposition (x ~ x8 + xr, W ~ w8 + wr, keeping the three first-order
    products) giving ~bf16 accuracy at fp8-DR speed.  Weights are scaled x8
    on host so fp8 quantization stays in e4m3 normal range.
  - Scores: one DoubleRow matmul per (head, key-tile) computes q.k AND adds
    the attention mask: slice 0 contracts the zero-padded per-head q against
    the two-head kT tile; slice 1 multiplies a 128*I identity into a {0,-120}
    mask tile.  PSUM gets qk - 15360*(1-m), i.e. s - 30*(1-m) after the 2^-9
    exp() activation scale (which also folds 1/sqrt(HD) and the x8 weight
    scales).  exp(s-30) underflows to exactly 0, matching the reference mask.
  - p@V runs with exp tiles (fp16) as the stationary operand: output lands
    as [q, 64v+1ones] per head with only 65 moving rows per instruction, and
    softmax denominators are normalized by a per-partition reciprocal via
    tensor_scalar.  A cheap PE transpose pass restores the [d, q] layout for
    the bf16 output projection.
Host side only reshapes/quantizes inputs and concatenates outputs.
"""

import numpy as np
import concourse.bass as bass
import concourse.mybir as mybir
from concourse import bacc
from concourse.dt import dt as cdt
from concourse.tile import TileContext
from concourse.bass_utils import run_bass_kernel_spmd

B, S, D, H, HD = 2, 2048, 512, 8, 64
P = 128
NCORES = 8
RPB = 4              # q-row blocks per batch
QB = S // RPB        # 512 query rows per core
NDC = D // P         # 4 chunks of the model dim
NKT = S // P         # 16 key tiles
KTG = 2              # key tiles per scores/exp group
NG = NKT // KTG      # 8 groups
VW = HD + 1          # 65 columns per head in VE (64 v + 1 ones)
NQB = QB // P        # 4 query-partition blocks

WS = 8.0             # host-side weight scale (keeps fp8 weights normal)
EXP_SCALE = 1.0 / 512.0   # 2^-9: undoes WS*WS and applies 1/sqrt(HD)
MASKV = -120.0       # mask tile value for masked-out entries
IDENTV = 128.0       # identity scale; IDENTV*MASKV*EXP_SCALE = -30
ONESV = 8.0          # V ones-column value (den = 8*sum(p); num = 8*sum(p*v))

# Schraudolph fp16 exp for the DVE/Pool engines: exp(psum/512) ~=
# bitcast_f16(int16(round(psum * 2*log2(e) + 15300))).  One tensor_scalar
# (mult, add) with an int16 output gives the rounded bits; the int16 tile is
# bitcast to f16 when consumed.  Max relative error ~3.3% on a sawtooth;
# applied to half the score tiles it costs ~1.5e-3 extra end-to-end error.
SCH_A = 2 * 1.4426950408889634
SCH_B = 15300.0

f32 = mybir.dt.float32
f32r = mybir.dt.float32r
bf16 = mybir.dt.bfloat16
f16 = mybir.dt.float16
f8 = mybir.dt.float8e4
u32 = mybir.dt.uint32
i16 = mybir.dt.int16
Alu = mybir.AluOpType
ActF = mybir.ActivationFunctionType
DR = mybir.MatmulPerfMode.DoubleRow

QM_W = (NKT + H // 2) * QB         # reversed mask tiles + 4 q slots
KI_W = P + S                       # fp8 identity + one d-chunk of kts


def _build_nc():
    nc = bacc.Bacc("TRN2", target_bir_lowering=False, debug=False,
                   num_devices=NCORES)
    dram = {}
    for nm, shape, dt_ in [
        ("xq8", [P, NDC, QB], f8), ("xqr", [P, NDC, QB], f8),
        ("xk8a", [P, NDC, QB], f8), ("xk8b", [P, NDC, S - QB], f8),
        ("xkra", [P, NDC, QB], f8), ("xkrb", [P, NDC, S - QB], f8),
        ("xv8", [P, NDC, S], f8), ("xvr", [P, NDC, S], f8),
        ("m_d", [P, NKT, QB], f8),
        ("w18", [P, NDC, D], f8), ("w1r", [P, NDC, D], f8),
        ("w28", [P, NDC, D], f8), ("w2r", [P, NDC, D], f8),
        ("w38", [P, NDC, D], f8), ("w3r", [P, NDC, D], f8),
        ("wo_d", [P, NDC, D], bf16),
        ("id8_d", [P, P], f8), ("idb_d", [P, P], bf16),
        ("b1_d", [P, NDC], f32), ("b2_d", [P, NDC], f32),
        ("bo_d", [1, D], f32),
    ]:
        dram[nm] = nc.dram_tensor(nm, shape, dt_, kind="ExternalInput")
    y = nc.dram_tensor("y", [QB, D], f32, kind="ExternalOutput")

    with TileContext(nc) as tc, nc.allow_low_precision("fp8 attention"):
        with (
            tc.tile_pool(name="persist", bufs=1) as pp,
            tc.tile_pool(name="small", bufs=1) as sp,
            tc.tile_pool(name="ps_s", bufs=3, space="PSUM") as ps_s,
            tc.tile_pool(name="ps_a", bufs=2, space="PSUM") as ps_a,
            tc.tile_pool(name="pex", bufs=34) as pex,
            tc.tile_pool(name="psx", bufs=5) as psx,
            tc.tile_pool(name="yp", bufs=2) as yp,
        ):
            # ---- persistent SBUF tiles ----
            KI = [pp.tile([P, KI_W], f8, tag=f"KI{m}", name=f"KI{m}")
                  for m in range(1, NDC)]
            KI0 = [pp.tile([P, P + QB], f8, tag=f"KI0_{kb}",
                           name=f"KI0_{kb}") for kb in range(S // QB)]
            QM = [pp.tile([P, QM_W], f8, tag=f"QM{i}", name=f"QM{i}")
                  for i in range(2)]
            VE = pp.tile([P, NKT * H * VW], f16, tag="VE", name="VE")
            sb = {}
            for nm, w, dt_ in [
                ("xq8", NDC * QB, f8), ("xqr", NDC * QB, f8),
                ("xk8a", NDC * QB, f8), ("xk8b", NDC * (S - QB), f8),
                ("xkra", NDC * QB, f8), ("xkrb", NDC * (S - QB), f8),
                ("xv8", NDC * S, f8), ("xvr", NDC * S, f8),
                ("w18", NDC * D, f8), ("w1r", NDC * D, f8),
                ("w28", NDC * D, f8), ("w2r", NDC * D, f8),
                ("w38", NDC * D, f8), ("w3r", NDC * D, f8),
                ("wo_d", NDC * D, bf16), ("idb_d", P, bf16),
            ]:
                inner = w // (NDC if nm not in ("idb_d",) else 1)
                if nm == "idb_d":
                    sb[nm] = pp.tile([P, P], dt_, tag=nm, name=nm)
                else:
                    sb[nm] = pp.tile([P, NDC, inner], dt_, tag=nm, name=nm)
            out2 = [pp.tile([P, D], bf16, tag=f"o2_{qb}", name=f"o2_{qb}")
                    for qb in range(NQB)]
            outT = [pp.tile([P, QB], bf16, tag=f"oT{m}", name=f"oT{m}")
                    for m in range(NDC)]

            b1sb = sp.tile([P, NDC], f32, tag="b1sb", name="b1sb")
            b2sb = sp.tile([P, NDC], f32, tag="b2sb", name="b2sb")
            borow = sp.tile([1, D], f32r, tag="borow", name="borow")
            bob = sp.tile([P, D], f32, tag="bob", name="bob")
            ones_r = sp.tile([1, P], f32r, tag="ones_r", name="ones_r")
            nc.vector._memset_packed(ones_r[:].bitcast(u32), 0x3F800000)

            # bulk DMAs split across SP and Pool rings; Q/K/mask inputs
            # first so the attention pipeline can start early
            for nm in ["xq8", "w18", "xqr"]:
                nc.sync.dma_start(sb[nm][:], dram[nm][:])
            nc.sync.dma_start(sb["xk8a"][:], dram["xk8a"][:])
            nc.sync.dma_start(sb["w28"][:], dram["w28"][:])
            nc.sync.dma_start(b1sb[:], dram["b1_d"][:])
            nc.sync.dma_start(sb["xk8b"][:], dram["xk8b"][:])
            for kb in range(S // QB):
                nc.sync.dma_start(KI0[kb][:, 0:P], dram["id8_d"][:])
            for m in range(NDC - 1):
                nc.sync.dma_start(KI[m][:, 0:P], dram["id8_d"][:])
            nc.sync.dma_start(sb["wo_d"][:], dram["wo_d"][:])
            nc.sync.dma_start(borow[:], dram["bo_d"][:].squeeze().bitcast(f32r))
            nc.sync.dma_start(sb["idb_d"][:], dram["idb_d"][:])
            nc.gpsimd.dma_start(b2sb[:], dram["b2_d"][:])
            nc.gpsimd.dma_start(sb["xkra"][:], dram["xkra"][:])
            nc.gpsimd.dma_start(sb["w2r"][:], dram["w2r"][:])
            nc.gpsimd.dma_start(sb["w1r"][:], dram["w1r"][:])
            nc.gpsimd.dma_start(
                QM[0][:, (NKT // 2) * QB:NKT * QB].rearrange(
                    "p (t q) -> p t q", t=NKT // 2),
                dram["m_d"][:, NKT // 2:, :])
            nc.gpsimd.dma_start(
                QM[0][:, 0:(NKT // 2) * QB].rearrange(
                    "p (t q) -> p t q", t=NKT // 2),
                dram["m_d"][:, 0:NKT // 2, :])
            nc.sync.dma_start(
                QM[1][:, 0:NKT * QB].rearrange(
                    "p (t q) -> p t q", t=NKT), dram["m_d"][:])
            for nm in ["xkrb", "xv8", "w38", "xvr", "w3r"]:
                nc.gpsimd.dma_start(sb[nm][:], dram[nm][:])

            # zero the per-head q slots (conversions fill 64 rows per slot)
            qmz = QM[0][:, NKT * QB:].bitcast(u32)
            nc.scalar.mul(qmz, qmz, 0.0)
            qmz1 = QM[1][:, NKT * QB:].bitcast(u32)
            nc.vector._memset_packed(qmz1, 0)
            # V ones-columns
            VEv = VE[:].rearrange("p (t h c) -> p t h c", t=NKT, c=VW)
            nc.gpsimd.memset(VEv[:, :, :, HD:VW], ONESV)

            def proj_dr(ps_ap, wmain, wres, xmain, xres, mcols, xcols):
                """psum += (w8+wr).T x8 + w8.T xr over 4 chunks, 3 DR mms."""
                pairs = [(wmain, xmain), (wmain, xres), (wres, xmain)]
                for i, (wt, xt) in enumerate(pairs):
                    for j in range(NDC // 2):
                        nc.tensor.matmul(
                            ps_ap,
                            wt[:, 2 * j:2 * j + 2, mcols],
                            xt[:, 2 * j:2 * j + 2, xcols],
                            start=(i == 0 and j == 0),
                            stop=(i == 2 and j == NDC // 2 - 1),
                            perf_mode=DR)

            def emit_q_proj(m):
                # Q projection chunk -> zero-padded per-head fp8 slots in QM
                ps = psp.tile([P, QB], f32, tag="ps", name="psq")
                proj_dr(ps[:], sb["w18"], sb["w1r"], sb["xq8"], sb["xqr"],
                        slice(m * P, (m + 1) * P), slice(None))
                for hp in range(2):
                    h = 2 * m + hp
                    base = hp * HD
                    nc.vector.tensor_scalar(
                        QM[base:base + HD,
                           (NKT + h) * QB:(NKT + h + 1) * QB],
                        ps[base:base + HD, :],
                        b1sb[base:base + HD, m:m + 1], None, Alu.add)

            def emit_k_proj(m):
                # K projection chunk -> fp8 kts slot m in KI.  The m=0
                # conversions run on the (otherwise idle) scalar engine so
                # the startup isn't serialized behind the DVE queue.
                for kb in range(S // QB):
                    ps = ps_a.tile([P, QB], f32, tag=f"pv{kb % 2}",
                                   name="psk", bufs=1)
                    if kb == 0:
                        proj_dr(ps[:], sb["w28"], sb["w2r"],
                                sb["xk8a"], sb["xkra"],
                                slice(m * P, (m + 1) * P), slice(None))
                    else:
                        proj_dr(ps[:], sb["w28"], sb["w2r"],
                                sb["xk8b"], sb["xkrb"],
                                slice(m * P, (m + 1) * P),
                                slice((kb - 1) * QB, kb * QB))
                    if m == 0:
                        dst = KI0[kb][:, P:]
                        if kb == 0:
                            nc.scalar.add(dst, ps[:], b2sb[:, m:m + 1])
                        else:
                            nc.vector.tensor_scalar(
                                dst, ps[:], b2sb[:, m:m + 1], None, Alu.add)
                    elif m == 1:
                        dst = KI[m - 1][:, P + kb * QB: P + (kb + 1) * QB]
                        nc.vector.tensor_scalar(
                            dst, ps[:], b2sb[:, m:m + 1], None, Alu.add)
                    else:
                        dst = KI[m - 1][:, P + kb * QB: P + (kb + 1) * QB]
                        nc.scalar.add(dst, ps[:], b2sb[:, m:m + 1])

            VEv = VE[:].rearrange("p (t h c) -> p t h c", t=NKT, c=VW)

            def emit_v_proj(t0, t1):
                # V projection tiles -> fp16 VE (head-interleaved; the v bias
                # b3 is folded into the output-projection bias on the host,
                # so this is a pure convert, split across ACT and DVE)
                for t in range(t0, t1):
                    ps = ps_a.tile([P, D], f32, tag=f"pv{t % 2}",
                                   name="psv", bufs=1)
                    pairs = [(sb["xv8"], sb["w38"]), (sb["xvr"], sb["w38"]),
                             (sb["xv8"], sb["w3r"])]
                    for i, (xt, wt) in enumerate(pairs):
                        for j in range(NDC // 2):
                            nc.tensor.matmul(
                                ps[:],
                                xt[:, 2 * j:2 * j + 2, t * P:(t + 1) * P],
                                wt[:, 2 * j:2 * j + 2, :],
                                start=(i == 0 and j == 0),
                                stop=(i == 2 and j == NDC // 2 - 1),
                                perf_mode=DR)
                    src = ps[:].rearrange("p (h d) -> p h d", d=HD)
                    nc.scalar.copy(VEv[:, t, :, 0:HD], src)

            # ==== attention building blocks ====
            KIv = [KI[m][:].rearrange("p (s c) -> p s c", c=P)
                   for m in range(NDC - 1)]
            KI0v = [KI0[kb][:].rearrange("p (s c) -> p s c", c=P)
                    for kb in range(S // QB)]
            QMv = [QM[i][:].rearrange("p (s q) -> p s q", q=QB)
                   for i in range(2)]

            # exp engine per (head, group): A = exact exp on ACT; V =
            # Schraudolph int16 trick on DVE; P = two-stage lane for the
            # PSUM-blind Pool engine (ACT/DVE copies the f32 scores to an
            # f16 staging tile, Pool runs the Schraudolph from SBUF).  The
            # DVE/Pool share ramps up as projection/DMA duties wind down.
            EXP_ENG = {0: "AAAAAAAA", 1: "AAVAPAVA", 2: "APVAPAVP"}
            EXP_ENG_LATE = "APVPAPVP"

            def emit_scores_exp(h):
                m = h // 2
                engs = EXP_ENG.get(h, EXP_ENG_LATE)
                pes = []
                for g in range(NG):
                    sg = ps_s.tile([P, KTG * QB], f32, tag="s", name="s")
                    for j in range(KTG):
                        t = g * KTG + j
                        # slice 0: (identity, reversed-mask tile)
                        # slice 1: (kT tile, zero-padded per-head q)
                        if m == 0:
                            kv = KI0v[t // 4]
                            ks = 1 + t % 4
                        else:
                            kv = KIv[m - 1]
                            ks = 1 + t
                        ms = NKT - 1 - t
                        dq = NKT + h % 4 - ms
                        nc.tensor.matmul(
                            sg[:, j * QB:(j + 1) * QB],
                            kv[:, 0:ks + 1:ks, :],
                            QMv[h // 4][:, ms:ms + dq + 1:dq, :],
                            start=True, stop=True, perf_mode=DR)
                    if engs[g] == "A":
                        pe = pex.tile([P, KTG * QB], f16, tag="pe", name="pe")
                        nc.scalar.activation(pe[:], sg[:], ActF.Exp,
                                             scale=EXP_SCALE)
                        pes.append(pe[:])
                    elif engs[g] == "V":
                        pe = pex.tile([P, KTG * QB], i16, tag="pe", name="pe")
                        nc.vector.tensor_scalar(pe[:], sg[:], SCH_A, SCH_B,
                                                Alu.mult, op1=Alu.add)
                        pes.append(pe[:].bitcast(f16))
                    else:
                        st = psx.tile([P, KTG * QB], f16, tag="st", name="st")
                        if g % 2 == 0:
                            nc.scalar.copy(st[:], sg[:])
                        else:
                            nc.vector.tensor_copy(st[:], sg[:])
                        pe = pex.tile([P, KTG * QB], i16, tag="pe", name="pe")
                        nc.gpsimd.tensor_scalar(pe[:], st[:], SCH_A, SCH_B,
                                                Alu.mult, op1=Alu.add)
                        pes.append(pe[:].bitcast(f16))
                return pes

            def emit_pv(h, pes, t_outer=False):
                # p @ [V | ones]: p tiles stationary -> [q, 65] outputs
                if t_outer:
                    # last head: qb2/qb3 ride retired score-pool banks and
                    # pre-accumulate t<15; qb0/qb1 interleave by t; only
                    # four t=15 matmuls depend on the final exp
                    pvs = [ps_a.tile([P, VW], f32, tag=f"pv{qb}",
                                     name="pv", bufs=1) for qb in range(2)]
                    pvs += [ps_s.tile([P, VW], f32, tag="s", name="pvs")
                            for _ in range(2)]
                    for qb in (2, 3):
                        for t in range(NKT - 1):
                            g, j = divmod(t, KTG)
                            nc.tensor.matmul(
                                pvs[qb][:],
                                pes[g][:, j * QB + qb * P:
                                       j * QB + (qb + 1) * P],
                                VEv[:, t, h, :],
                                start=(t == 0), stop=False)
                    for t in range(NKT):
                        g, j = divmod(t, KTG)
                        for qb in range(2):
                            nc.tensor.matmul(
                                pvs[qb][:],
                                pes[g][:, j * QB + qb * P:
                                       j * QB + (qb + 1) * P],
                                VEv[:, t, h, :],
                                start=(t == 0), stop=(t == NKT - 1))
                    t, (g, j) = NKT - 1, divmod(NKT - 1, KTG)
                    for qb in (2, 3):
                        nc.tensor.matmul(
                            pvs[qb][:],
                            pes[g][:, j * QB + qb * P:
                                   j * QB + (qb + 1) * P],
                            VEv[:, t, h, :],
                            start=False, stop=True)
                for qb in range(NQB):
                    if t_outer:
                        pv = pvs[qb]
                    else:
                        pv = ps_a.tile([P, VW], f32, tag=f"pv{qb % 2}",
                                       name="pv", bufs=1)
                        for t in range(NKT):
                            g, j = divmod(t, KTG)
                            nc.tensor.matmul(
                                pv[:],
                                pes[g][:, j * QB + qb * P:
                                       j * QB + (qb + 1) * P],
                                VEv[:, t, h, :],
                                start=(t == 0), stop=(t == NKT - 1))
                    rcp = yp.tile([P, 1], f32, tag="rcp", name="rcp", bufs=2)
                    nc.vector.reciprocal(rcp[:], pv[:, HD:VW])
                    nc.vector.tensor_scalar(
                        out2[qb][:, h * HD:(h + 1) * HD],
                        pv[:, 0:HD], rcp[:], None, Alu.mult)

            oTT = pp.tile([P, NDC, QB], bf16, tag="oTT", name="oTT")

            def emit_transpose(m):
                # transpose out2[:, d-chunk m] -> oTT[:, m, :] ([d, q] layout)
                pst = ps_a.tile([P, QB], bf16, tag=f"pv{m % 2}",
                                name="pst", bufs=1)
                for qt in range(NQB):
                    nc.tensor.matmul(
                        pst[:, qt * P:(qt + 1) * P],
                        out2[qt][:, m * P:(m + 1) * P],
                        sb["idb_d"][:],
                        start=True, stop=True, is_transpose=True)
                nc.vector.tensor_copy(oTT[:, m, :], pst[:])

            ypb = [pp.tile([P, D], f32, tag=f"ypb{qt}", name=f"ypb{qt}")
                   for qt in range(NQB)]

            def emit_tail_early(qt):
                # partial output projection over d-chunks 0..2 (+ bias)
                ps = ps_a.tile([P, D], f32, tag=f"pv{qt % 2}",
                               name="psy", bufs=1)
                for m in range(NDC - 1):
                    nc.tensor.matmul(
                        ps[:], oTT[:, m, qt * P:(qt + 1) * P],
                        sb["wo_d"][:, m, :],
                        start=(m == 0), stop=(m == NDC - 2))
                nc.vector.tensor_tensor(ypb[qt][:], ps[:], bob[:],
                                        op=Alu.add)

            def emit_tail_late(qt):
                # last d-chunk + partial sum + store for one q block; the
                # y DMAs ride the SP and (idle by now) ACT hwdge queues
                ps = ps_a.tile([P, D], f32, tag=f"pv{qt % 2}",
                               name="psy", bufs=1)
                nc.tensor.matmul(
                    ps[:], oTT[:, NDC - 1, qt * P:(qt + 1) * P],
                    sb["wo_d"][:, NDC - 1, :], start=True, stop=True)
                ysb = yp.tile([P, D], f32, tag="ysb", name="ysb", bufs=2)
                nc.vector.tensor_tensor(ysb[:], ps[:], ypb[qt][:],
                                        op=Alu.add)
                eng = nc.sync if qt % 2 == 0 else nc.scalar
                eng.dma_start(y[qt * P:(qt + 1) * P, :], ysb[:])

            # ==== PE p-state warm-up ====
            wps = ps_a.tile([P, D], f32, tag="pv0", name="wps", bufs=1)
            for w in range(15):
                nc.tensor.matmul(wps[:, 0:P], ones_r[:], ones_r[:, 0:P],
                                 start=True, stop=True)

            # ==== software-pipelined emission ====
            # PE is in-order: feed the activation engine (the critical
            # resource) as early and as continuously as possible.
            # Q projection pairs share wide psum tiles so the conversion
            # chain is not serialized behind the "ps" ring.
            for mp in [0]:
                psq = ps_s.tile([P, KTG * QB], f32, tag="s", name="psq")
                for mh in range(2):
                    m = 2 * mp + mh
                    proj_dr(psq[:, mh * QB:(mh + 1) * QB],
                            sb["w18"], sb["w1r"], sb["xq8"], sb["xqr"],
                            slice(m * P, (m + 1) * P), slice(None))
                for mh in range(2):
                    m = 2 * mp + mh
                    for hp in range(2):
                        h = 2 * m + hp
                        base = hp * HD
                        nc.vector.tensor_scalar(
                            QM[h // 4][base:base + HD,
                                       (NKT + h % 4) * QB:
                                       (NKT + h % 4 + 1) * QB],
                            psq[base:base + HD, mh * QB:(mh + 1) * QB],
                            b1sb[base:base + HD, m:m + 1], None, Alu.add)
            emit_k_proj(0)
            for mp in [1]:
                psq = ps_s.tile([P, KTG * QB], f32, tag="s", name="psq")
                for mh in range(2):
                    m = 2 * mp + mh
                    proj_dr(psq[:, mh * QB:(mh + 1) * QB],
                            sb["w18"], sb["w1r"], sb["xq8"], sb["xqr"],
                            slice(m * P, (m + 1) * P), slice(None))
                for mh in range(2):
                    m = 2 * mp + mh
                    for hp in range(2):
                        h = 2 * m + hp
                        base = hp * HD
                        nc.vector.tensor_scalar(
                            QM[h // 4][base:base + HD,
                                       (NKT + h % 4) * QB:
                                       (NKT + h % 4 + 1) * QB],
                            psq[base:base + HD, mh * QB:(mh + 1) * QB],
                            b1sb[base:base + HD, m:m + 1], None, Alu.add)
            sc = {0: emit_scores_exp(0)}
            emit_k_proj(1)
            sc[1] = emit_scores_exp(1)
            emit_k_proj(2)
            sc[2] = emit_scores_exp(2)
            # broadcast bo across partitions via a K=1 matmul (emitted here
            # so its late DMA never gates the Q/K projections at the head of
            # the in-order PE stream)
            psb2 = ps_a.tile([P, D], f32, tag="pv1", name="psb2", bufs=1)
            nc.tensor.matmul(psb2[:], ones_r[:], borow[:], start=True,
                             stop=True)
            nc.vector.tensor_copy(bob[:], psb2[:])
            emit_v_proj(0, 8)
            emit_k_proj(3)
            sc[3] = emit_scores_exp(3)
            emit_v_proj(8, 16)
            emit_pv(0, sc.pop(0))
            sc[4] = emit_scores_exp(4)
            emit_pv(1, sc.pop(1))
            sc[5] = emit_scores_exp(5)
            emit_pv(2, sc.pop(2))
            sc[6] = emit_scores_exp(6)
            emit_pv(3, sc.pop(3))
            sc[7] = emit_scores_exp(7)
            emit_pv(4, sc.pop(4))
            emit_pv(5, sc.pop(5))
            emit_transpose(0)
            emit_transpose(1)
            emit_pv(6, sc.pop(6))
            emit_transpose(2)
            for qt in range(NQB):
                emit_tail_early(qt)
            emit_pv(7, sc.pop(7), t_outer=True)
            for qt in range(NQB):
                pst = ps_a.tile([P, P], bf16, tag=f"pv{qt % 2}",
                                name="pst3", bufs=1)
                nc.tensor.matmul(
                    pst[:], out2[qt][:, (NDC - 1) * P:], sb["idb_d"][:],
                    start=True, stop=True, is_transpose=True)
                nc.vector.tensor_copy(
                    oTT[:, NDC - 1, qt * P:(qt + 1) * P], pst[:])
                emit_tail_late(qt)

    nc.finalize()
    return nc


_CACHE = {}


def _get_nc():
    if "nc" not in _CACHE:
        _CACHE["nc"] = _build_nc()
    return _CACHE["nc"]


F8NP = cdt.np(cdt.float8e4)
BF16NP = cdt.np(cdt.bfloat16)


def _to_chunked(a, inner):
    """[D, N] -> [P, NDC, N] with row d = c*P+p mapped to [p, c, :]."""
    return np.ascontiguousarray(
        a.reshape(NDC, P, inner).transpose(1, 0, 2))


def _split8(a):
    """Double-fp8 decomposition: a ~ hi + lo with both parts fp8e4."""
    hi = a.astype(F8NP)
    lo = (a - hi.astype(np.float32)).astype(F8NP)
    return hi, lo


def _prep_shared(W1, b1, W2, b2, W3, b3, Wo, bo):
    f = np.float32
    id8 = np.zeros((P, P), f)
    np.fill_diagonal(id8, IDENTV)
    idb = np.zeros((P, P), f)
    np.fill_diagonal(idb, 1.0)
    shared = {
        "id8_d": id8.astype(F8NP),
        "idb_d": idb.astype(BF16NP),
        "wo_d": _to_chunked(np.ascontiguousarray(np.asarray(Wo, f).T),
                            D).astype(BF16NP),
        "b1_d": np.ascontiguousarray(
            (np.asarray(b1, f) * f(WS)).reshape(NDC, P).T),
        "b2_d": np.ascontiguousarray(
            (np.asarray(b2, f) * f(WS)).reshape(NDC, P).T),
        # v-bias folds through the attention average and the output
        # projection exactly: y = (attn + b3) @ Wo.T + bo
        "bo_d": (np.asarray(bo, f)
                 + np.asarray(b3, f) @ np.asarray(Wo, f).T
                 ).reshape(1, D).copy(),
    }
    for nm, W in [("w1", W1), ("w2", W2), ("w3", W3)]:
        hi, lo = _split8(_to_chunked(np.asarray(W, f).T * f(WS), D))
        shared[nm + "8"] = hi
        shared[nm + "r"] = lo
    return shared


def build_in_maps(q_in, k_in, v_in, mask, W1, b1, W2, b2, W3, b3, Wo, bo):
    f = np.float32
    q_in = np.asarray(q_in, f)
    k_in = np.asarray(k_in, f)
    v_in = np.asarray(v_in, f)
    mask = np.asarray(mask)
    shared = _prep_shared(W1, b1, W2, b2, W3, b3, Wo, bo)
    kc = [_split8(_to_chunked(k_in[b].T, S)) for b in range(B)]
    kc = [(np.ascontiguousarray(h[:, :, :QB]),
           np.ascontiguousarray(h[:, :, QB:]),
           np.ascontiguousarray(l[:, :, :QB]),
           np.ascontiguousarray(l[:, :, QB:])) for h, l in kc]
    vc = [_split8(_to_chunked(v_in[b].T, S)) for b in range(B)]
    in_maps = []
    for c in range(NCORES):
        b, r = divmod(c, RPB)
        sl = slice(r * QB, (r + 1) * QB)
        # masked (m==0) entries get MASKV (=-120), unmasked get 0
        mt = (f(1.0) - mask[b, 0, sl, :].T.astype(f)) * f(MASKV)
        q8, qr = _split8(_to_chunked(
            np.ascontiguousarray(q_in[b, sl, :].T), QB))
        in_maps.append({
            "xq8": q8, "xqr": qr,
            "xk8a": kc[b][0], "xk8b": kc[b][1],
            "xkra": kc[b][2], "xkrb": kc[b][3],
            "xv8": vc[b][0], "xvr": vc[b][1],
            "m_d": np.ascontiguousarray(
                mt.reshape(NKT, P, QB)[::-1].transpose(1, 0, 2)).astype(F8NP),
            **shared,
        })
    return in_maps


def kernel(q_in, k_in, v_in, mask, W1, b1, W2, b2, W3, b3, Wo, bo):
    f = np.float32
    nc = _get_nc()
    in_maps = build_in_maps(q_in, k_in, v_in, mask, W1, b1, W2, b2, W3, b3,
                            Wo, bo)
    res = run_bass_kernel_spmd(nc, in_maps, list(range(NCORES)))
    out = np.empty((B, S, D), f)
    for c in range(NCORES):
        b, r = divmod(c, RPB)
        out[b, r * QB:(r + 1) * QB, :] = res.results[c]["y"]
    return out



# revision 32
# speedup vs baseline: 1.0483x; 1.0483x over previous
"""Multi-head attention on 8 Trainium2 NeuronCores — fp8 DoubleRow version.

Sharding: batch (2) x query-row-block (4) -> 8 cores; each core computes full
attention for its 512 query rows of one batch (K/V projected for all keys).

Techniques vs the fp32r baseline:
  - QKV projections run as fp8e4 DoubleRow matmuls on a host-side double-fp8
    decomposition (x ~ x8 + xr, W ~ w8 + wr, keeping the three first-order
    products) giving ~bf16 accuracy at fp8-DR speed.  Weights are scaled x8
    on host so fp8 quantization stays in e4m3 normal range.
  - Scores: one DoubleRow matmul per (head, key-tile) computes q.k AND adds
    the attention mask: slice 0 contracts the zero-padded per-head q against
    the two-head kT tile; slice 1 multiplies a 128*I identity into a {0,-120}
    mask tile.  PSUM gets qk - 15360*(1-m), i.e. s - 30*(1-m) after the 2^-9
    exp() activation scale (which also folds 1/sqrt(HD) and the x8 weight
    scales).  exp(s-30) underflows to exactly 0, matching the reference mask.
  - p@V runs with exp tiles (fp16) as the stationary operand: output lands
    as [q, 64v+1ones] per head with only 65 moving rows per instruction, and
    softmax denominators are normalized by a per-partition reciprocal via
    tensor_scalar.  A cheap PE transpose pass restores the [d, q] layout for
    the bf16 output projection.
Host side only reshapes/quantizes inputs and concatenates outputs.
"""

import numpy as np
import concourse.bass as bass
import concourse.mybir as mybir
from concourse import bacc
from concourse.dt import dt as cdt
from concourse.tile import TileContext
from concourse.bass_utils import run_bass_kernel_spmd

B, S, D, H, HD = 2, 2048, 512, 8, 64
P = 128
NCORES = 8
RPB = 4              # q-row blocks per batch
QB = S // RPB        # 512 query rows per core
NDC = D // P         # 4 chunks of the model dim
NKT = S // P         # 16 key tiles
KTG = 2              # key tiles per scores/exp group
NG = NKT // KTG      # 8 groups
VW = HD + 1          # 65 columns per head in VE (64 v + 1 ones)
NQB = QB // P        # 4 query-partition blocks

WS = 8.0             # host-side weight scale (keeps fp8 weights normal)
EXP_SCALE = 1.0 / 512.0   # 2^-9: undoes WS*WS and applies 1/sqrt(HD)
MASKV = -120.0       # mask tile value for masked-out entries
IDENTV = 128.0       # identity scale; IDENTV*MASKV*EXP_SCALE = -30
ONESV = 8.0          # V ones-column value (den = 8*sum(p); num = 8*sum(p*v))

# Schraudolph fp16 exp for the DVE/Pool engines: exp(psum/512) ~=
# bitcast_f16(int16(round(psum * 2*log2(e) + 15300))).  One tensor_scalar
# (mult, add) with an int16 output gives the rounded bits; the int16 tile is
# bitcast to f16 when consumed.  Max relative error ~3.3% on a sawtooth;
# applied to half the score tiles it costs ~1.5e-3 extra end-to-end error.
SCH_A = 2 * 1.4426950408889634
SCH_B = 15300.0

f32 = mybir.dt.float32
f32r = mybir.dt.float32r
bf16 = mybir.dt.bfloat16
f16 = mybir.dt.float16
f8 = mybir.dt.float8e4
u32 = mybir.dt.uint32
i16 = mybir.dt.int16
Alu = mybir.AluOpType
ActF = mybir.ActivationFunctionType
DR = mybir.MatmulPerfMode.DoubleRow

QM_W = (NKT + H // 2) * QB         # reversed mask tiles + 4 q slots
KI_W = P + S                       # fp8 identity + one d-chunk of kts


def _build_nc():
    nc = bacc.Bacc("TRN2", target_bir_lowering=False, debug=False,
                   num_devices=NCORES)
    dram = {}
    for nm, shape, dt_ in [
        ("xq8", [P, NDC, QB], f8), ("xqr", [P, NDC, QB], f8),
        ("xk8a", [P, NDC, QB], f8), ("xk8b", [P, NDC, S - QB], f8),
        ("xkra", [P, NDC, QB], f8), ("xkrb", [P, NDC, S - QB], f8),
        ("xv8", [P, NDC, S], f8), ("xvr", [P, NDC, S], f8),
        ("m_d", [P, NKT, QB], f8),
        ("w18", [P, NDC, D], f8), ("w1r", [P, NDC, D], f8),
        ("w28", [P, NDC, D], f8), ("w2r", [P, NDC, D], f8),
        ("w38", [P, NDC, D], f8), ("w3r", [P, NDC, D], f8),
        ("wo_d", [P, NDC, D], bf16),
        ("id8_d", [P, P], f8), ("idb_d", [P, P], bf16),
        ("b1_d", [P, NDC], f32), ("b2_d", [P, NDC], f32),
        ("bo_d", [1, D], f32),
    ]:
        dram[nm] = nc.dram_tensor(nm, shape, dt_, kind="ExternalInput")
    y = nc.dram_tensor("y", [QB, D], f32, kind="ExternalOutput")

    with TileContext(nc) as tc, nc.allow_low_precision("fp8 attention"):
        with (
            tc.tile_pool(name="persist", bufs=1) as pp,
            tc.tile_pool(name="small", bufs=1) as sp,
            tc.tile_pool(name="ps_s", bufs=3, space="PSUM") as ps_s,
            tc.tile_pool(name="ps_a", bufs=2, space="PSUM") as ps_a,
            tc.tile_pool(name="pex", bufs=34) as pex,
            tc.tile_pool(name="psx", bufs=5) as psx,
            tc.tile_pool(name="yp", bufs=2) as yp,
        ):
            # ---- persistent SBUF tiles ----
            KI = [pp.tile([P, KI_W], f8, tag=f"KI{m}", name=f"KI{m}")
                  for m in range(1, NDC)]
            KI0 = [pp.tile([P, P + QB], f8, tag=f"KI0_{kb}",
                           name=f"KI0_{kb}") for kb in range(S // QB)]
            QM = [pp.tile([P, QM_W], f8, tag=f"QM{i}", name=f"QM{i}")
                  for i in range(2)]
            VE = pp.tile([P, NKT * H * VW], f16, tag="VE", name="VE")
            sb = {}
            for nm, w, dt_ in [
                ("xq8", NDC * QB, f8), ("xqr", NDC * QB, f8),
                ("xk8a", NDC * QB, f8), ("xk8b", NDC * (S - QB), f8),
                ("xkra", NDC * QB, f8), ("xkrb", NDC * (S - QB), f8),
                ("xv8", NDC * S, f8), ("xvr", NDC * S, f8),
                ("w18", NDC * D, f8), ("w1r", NDC * D, f8),
                ("w28", NDC * D, f8), ("w2r", NDC * D, f8),
                ("w38", NDC * D, f8), ("w3r", NDC * D, f8),
                ("wo_d", NDC * D, bf16), ("idb_d", P, bf16),
            ]:
                inner = w // (NDC if nm not in ("idb_d",) else 1)
                if nm == "idb_d":
                    sb[nm] = pp.tile([P, P], dt_, tag=nm, name=nm)
                else:
                    sb[nm] = pp.tile([P, NDC, inner], dt_, tag=nm, name=nm)
            out2 = [pp.tile([P, D], bf16, tag=f"o2_{qb}", name=f"o2_{qb}")
                    for qb in range(NQB)]
            outT = [pp.tile([P, QB], bf16, tag=f"oT{m}", name=f"oT{m}")
                    for m in range(NDC)]

            b1sb = sp.tile([P, NDC], f32, tag="b1sb", name="b1sb")
            b2sb = sp.tile([P, NDC], f32, tag="b2sb", name="b2sb")
            borow = sp.tile([1, D], f32r, tag="borow", name="borow")
            bob = sp.tile([P, D], f32, tag="bob", name="bob")
            ones_r = sp.tile([1, P], f32r, tag="ones_r", name="ones_r")
            nc.vector._memset_packed(ones_r[:].bitcast(u32), 0x3F800000)

            # bulk DMAs split across SP and Pool rings; Q/K/mask inputs
            # first so the attention pipeline can start early
            for nm in ["xq8", "w18", "xqr"]:
                nc.sync.dma_start(sb[nm][:], dram[nm][:])
            nc.sync.dma_start(sb["xk8a"][:], dram["xk8a"][:])
            nc.sync.dma_start(sb["w28"][:], dram["w28"][:])
            nc.sync.dma_start(b1sb[:], dram["b1_d"][:])
            nc.sync.dma_start(sb["xk8b"][:], dram["xk8b"][:])
            for kb in range(S // QB):
                nc.sync.dma_start(KI0[kb][:, 0:P], dram["id8_d"][:])
            for m in range(NDC - 1):
                nc.sync.dma_start(KI[m][:, 0:P], dram["id8_d"][:])
            nc.sync.dma_start(sb["wo_d"][:], dram["wo_d"][:])
            nc.sync.dma_start(borow[:], dram["bo_d"][:].squeeze().bitcast(f32r))
            nc.sync.dma_start(sb["idb_d"][:], dram["idb_d"][:])
            nc.gpsimd.dma_start(b2sb[:], dram["b2_d"][:])
            nc.gpsimd.dma_start(sb["xkra"][:], dram["xkra"][:])
            nc.gpsimd.dma_start(sb["w2r"][:], dram["w2r"][:])
            nc.gpsimd.dma_start(sb["w1r"][:], dram["w1r"][:])
            nc.gpsimd.dma_start(
                QM[0][:, (NKT // 2) * QB:NKT * QB].rearrange(
                    "p (t q) -> p t q", t=NKT // 2),
                dram["m_d"][:, NKT // 2:, :])
            nc.gpsimd.dma_start(
                QM[0][:, 0:(NKT // 2) * QB].rearrange(
                    "p (t q) -> p t q", t=NKT // 2),
                dram["m_d"][:, 0:NKT // 2, :])
            nc.sync.dma_start(
                QM[1][:, 0:NKT * QB].rearrange(
                    "p (t q) -> p t q", t=NKT), dram["m_d"][:])
            for nm in ["xkrb", "xv8", "w38", "xvr", "w3r"]:
                nc.gpsimd.dma_start(sb[nm][:], dram[nm][:])

            # zero the per-head q slots (conversions fill 64 rows per slot)
            qmz = QM[0][:, NKT * QB:].bitcast(u32)
            nc.scalar.mul(qmz, qmz, 0.0)
            qmz1 = QM[1][:, NKT * QB:].bitcast(u32)
            nc.vector._memset_packed(qmz1, 0)
            # V ones-columns
            VEv = VE[:].rearrange("p (t h c) -> p t h c", t=NKT, c=VW)
            nc.gpsimd.memset(VEv[:, :, :, HD:VW], ONESV)

            def proj_dr(ps_ap, wmain, wres, xmain, xres, mcols, xcols):
                """psum += (w8+wr).T x8 + w8.T xr over 4 chunks, 3 DR mms."""
                pairs = [(wmain, xmain), (wmain, xres), (wres, xmain)]
                for i, (wt, xt) in enumerate(pairs):
                    for j in range(NDC // 2):
                        nc.tensor.matmul(
                            ps_ap,
                            wt[:, 2 * j:2 * j + 2, mcols],
                            xt[:, 2 * j:2 * j + 2, xcols],
                            start=(i == 0 and j == 0),
                            stop=(i == 2 and j == NDC // 2 - 1),
                            perf_mode=DR)

            def emit_q_proj(m):
                # Q projection chunk -> zero-padded per-head fp8 slots in QM
                ps = psp.tile([P, QB], f32, tag="ps", name="psq")
                proj_dr(ps[:], sb["w18"], sb["w1r"], sb["xq8"], sb["xqr"],
                        slice(m * P, (m + 1) * P), slice(None))
                for hp in range(2):
                    h = 2 * m + hp
                    base = hp * HD
                    nc.vector.tensor_scalar(
                        QM[base:base + HD,
                           (NKT + h) * QB:(NKT + h + 1) * QB],
                        ps[base:base + HD, :],
                        b1sb[base:base + HD, m:m + 1], None, Alu.add)

            def emit_k_proj(m):
                # K projection chunk -> fp8 kts slot m in KI.  The m=0
                # conversions run on the (otherwise idle) scalar engine so
                # the startup isn't serialized behind the DVE queue.
                for kb in range(S // QB):
                    ps = ps_a.tile([P, QB], f32, tag=f"pv{kb % 2}",
                                   name="psk", bufs=1)
                    if kb == 0:
                        proj_dr(ps[:], sb["w28"], sb["w2r"],
                                sb["xk8a"], sb["xkra"],
                                slice(m * P, (m + 1) * P), slice(None))
                    else:
                        proj_dr(ps[:], sb["w28"], sb["w2r"],
                                sb["xk8b"], sb["xkrb"],
                                slice(m * P, (m + 1) * P),
                                slice((kb - 1) * QB, kb * QB))
                    if m == 0:
                        dst = KI0[kb][:, P:]
                        if kb == 0:
                            nc.scalar.add(dst, ps[:], b2sb[:, m:m + 1])
                        else:
                            nc.vector.tensor_scalar(
                                dst, ps[:], b2sb[:, m:m + 1], None, Alu.add)
                    elif m == 1:
                        dst = KI[m - 1][:, P + kb * QB: P + (kb + 1) * QB]
                        nc.vector.tensor_scalar(
                            dst, ps[:], b2sb[:, m:m + 1], None, Alu.add)
                    else:
                        dst = KI[m - 1][:, P + kb * QB: P + (kb + 1) * QB]
                        nc.scalar.add(dst, ps[:], b2sb[:, m:m + 1])

            VEv = VE[:].rearrange("p (t h c) -> p t h c", t=NKT, c=VW)

            def emit_v_proj(t0, t1):
                # V projection tiles -> fp16 VE (head-interleaved; the v bias
                # b3 is folded into the output-projection bias on the host,
                # so this is a pure convert, split across ACT and DVE)
                for t in range(t0, t1):
                    ps = ps_a.tile([P, D], f32, tag=f"pv{t % 2}",
                                   name="psv", bufs=1)
                    pairs = [(sb["xv8"], sb["w38"]), (sb["xvr"], sb["w38"]),
                             (sb["xv8"], sb["w3r"])]
                    for i, (xt, wt) in enumerate(pairs):
                        for j in range(NDC // 2):
                            nc.tensor.matmul(
                                ps[:],
                                xt[:, 2 * j:2 * j + 2, t * P:(t + 1) * P],
                                wt[:, 2 * j:2 * j + 2, :],
                                start=(i == 0 and j == 0),
                                stop=(i == 2 and j == NDC // 2 - 1),
                                perf_mode=DR)
                    src = ps[:].rearrange("p (h d) -> p h d", d=HD)
                    nc.scalar.copy(VEv[:, t, :, 0:HD], src)

            # ==== attention building blocks ====
            KIv = [KI[m][:].rearrange("p (s c) -> p s c", c=P)
                   for m in range(NDC - 1)]
            KI0v = [KI0[kb][:].rearrange("p (s c) -> p s c", c=P)
                    for kb in range(S // QB)]
            QMv = [QM[i][:].rearrange("p (s q) -> p s q", q=QB)
                   for i in range(2)]

            # exp engine per (head, group): A = exact exp on ACT; V =
            # Schraudolph int16 trick on DVE; P = two-stage lane for the
            # PSUM-blind Pool engine (ACT/DVE copies the f32 scores to an
            # f16 staging tile, Pool runs the Schraudolph from SBUF).  The
            # DVE/Pool share ramps up as projection/DMA duties wind down.
            EXP_ENG = {0: "AAAAAAAA", 1: "AAVAAVAA", 2: "AVAAVAAV"}
            EXP_ENG_LATE = "AVAVAVAV"

            def emit_scores_exp(h):
                m = h // 2
                engs = EXP_ENG.get(h, EXP_ENG_LATE)
                pes = []
                for g in range(NG):
                    sg = ps_s.tile([P, KTG * QB], f32, tag="s", name="s")
                    for j in range(KTG):
                        t = g * KTG + j
                        # slice 0: (identity, reversed-mask tile)
                        # slice 1: (kT tile, zero-padded per-head q)
                        if m == 0:
                            kv = KI0v[t // 4]
                            ks = 1 + t % 4
                        else:
                            kv = KIv[m - 1]
                            ks = 1 + t
                        ms = NKT - 1 - t
                        dq = NKT + h % 4 - ms
                        nc.tensor.matmul(
                            sg[:, j * QB:(j + 1) * QB],
                            kv[:, 0:ks + 1:ks, :],
                            QMv[h // 4][:, ms:ms + dq + 1:dq, :],
                            start=True, stop=True, perf_mode=DR)
                    if engs[g] == "A":
                        pe = pex.tile([P, KTG * QB], f16, tag="pe", name="pe")
                        nc.scalar.activation(pe[:], sg[:], ActF.Exp,
                                             scale=EXP_SCALE)
                        pes.append(pe[:])
                    elif engs[g] == "V":
                        pe = pex.tile([P, KTG * QB], i16, tag="pe", name="pe")
                        nc.vector.tensor_scalar(pe[:], sg[:], SCH_A, SCH_B,
                                                Alu.mult, op1=Alu.add)
                        pes.append(pe[:].bitcast(f16))
                    else:
                        st = psx.tile([P, KTG * QB], f16, tag="st", name="st")
                        if g % 2 == 0:
                            nc.scalar.copy(st[:], sg[:])
                        else:
                            nc.vector.tensor_copy(st[:], sg[:])
                        pe = pex.tile([P, KTG * QB], i16, tag="pe", name="pe")
                        nc.gpsimd.tensor_scalar(pe[:], st[:], SCH_A, SCH_B,
                                                Alu.mult, op1=Alu.add)
                        pes.append(pe[:].bitcast(f16))
                return pes

            def emit_pv(h, pes, t_outer=False):
                # p @ [V | ones]: p tiles stationary -> [q, 65] outputs
                if t_outer:
                    # last head: qb2/qb3 ride retired score-pool banks and
                    # pre-accumulate t<15; qb0/qb1 interleave by t; only
                    # four t=15 matmuls depend on the final exp
                    pvs = [ps_a.tile([P, VW], f32, tag=f"pv{qb}",
                                     name="pv", bufs=1) for qb in range(2)]
                    pvs += [ps_s.tile([P, VW], f32, tag="s", name="pvs")
                            for _ in range(2)]
                    for qb in (2, 3):
                        for t in range(NKT - 1):
                            g, j = divmod(t, KTG)
                            nc.tensor.matmul(
                                pvs[qb][:],
                                pes[g][:, j * QB + qb * P:
                                       j * QB + (qb + 1) * P],
                                VEv[:, t, h, :],
                                start=(t == 0), stop=False)
                    for t in range(NKT):
                        g, j = divmod(t, KTG)
                        for qb in range(2):
                            nc.tensor.matmul(
                                pvs[qb][:],
                                pes[g][:, j * QB + qb * P:
                                       j * QB + (qb + 1) * P],
                                VEv[:, t, h, :],
                                start=(t == 0), stop=(t == NKT - 1))
                    t, (g, j) = NKT - 1, divmod(NKT - 1, KTG)
                    for qb in (2, 3):
                        nc.tensor.matmul(
                            pvs[qb][:],
                            pes[g][:, j * QB + qb * P:
                                   j * QB + (qb + 1) * P],
                            VEv[:, t, h, :],
                            start=False, stop=True)
                for qb in range(NQB):
                    if t_outer:
                        pv = pvs[qb]
                    else:
                        pv = ps_a.tile([P, VW], f32, tag=f"pv{qb % 2}",
                                       name="pv", bufs=1)
                        for t in range(NKT):
                            g, j = divmod(t, KTG)
                            nc.tensor.matmul(
                                pv[:],
                                pes[g][:, j * QB + qb * P:
                                       j * QB + (qb + 1) * P],
                                VEv[:, t, h, :],
                                start=(t == 0), stop=(t == NKT - 1))
                    rcp = yp.tile([P, 1], f32, tag="rcp", name="rcp", bufs=2)
                    nc.vector.reciprocal(rcp[:], pv[:, HD:VW])
                    nc.vector.tensor_scalar(
                        out2[qb][:, h * HD:(h + 1) * HD],
                        pv[:, 0:HD], rcp[:], None, Alu.mult)

            oTT = pp.tile([P, NDC, QB], bf16, tag="oTT", name="oTT")

            def emit_transpose(m):
                # transpose out2[:, d-chunk m] -> oTT[:, m, :] ([d, q] layout)
                pst = ps_a.tile([P, QB], bf16, tag=f"pv{m % 2}",
                                name="pst", bufs=1)
                for qt in range(NQB):
                    nc.tensor.matmul(
                        pst[:, qt * P:(qt + 1) * P],
                        out2[qt][:, m * P:(m + 1) * P],
                        sb["idb_d"][:],
                        start=True, stop=True, is_transpose=True)
                nc.vector.tensor_copy(oTT[:, m, :], pst[:])

            ypb = [pp.tile([P, D], f32, tag=f"ypb{qt}", name=f"ypb{qt}")
                   for qt in range(NQB)]

            def emit_tail_early(qt):
                # partial output projection over d-chunks 0..2 (+ bias)
                ps = ps_a.tile([P, D], f32, tag=f"pv{qt % 2}",
                               name="psy", bufs=1)
                for m in range(NDC - 1):
                    nc.tensor.matmul(
                        ps[:], oTT[:, m, qt * P:(qt + 1) * P],
                        sb["wo_d"][:, m, :],
                        start=(m == 0), stop=(m == NDC - 2))
                nc.vector.tensor_tensor(ypb[qt][:], ps[:], bob[:],
                                        op=Alu.add)

            def emit_tail_late(qt):
                # last d-chunk + partial sum + store for one q block; the
                # y DMAs ride the SP and (idle by now) ACT hwdge queues
                ps = ps_a.tile([P, D], f32, tag=f"pv{qt % 2}",
                               name="psy", bufs=1)
                nc.tensor.matmul(
                    ps[:], oTT[:, NDC - 1, qt * P:(qt + 1) * P],
                    sb["wo_d"][:, NDC - 1, :], start=True, stop=True)
                ysb = yp.tile([P, D], f32, tag="ysb", name="ysb", bufs=2)
                nc.vector.tensor_tensor(ysb[:], ps[:], ypb[qt][:],
                                        op=Alu.add)
                eng = nc.sync if qt % 2 == 0 else nc.scalar
                eng.dma_start(y[qt * P:(qt + 1) * P, :], ysb[:])

            # ==== PE p-state warm-up ====
            wps = ps_a.tile([P, D], f32, tag="pv0", name="wps", bufs=1)
            for w in range(15):
                nc.tensor.matmul(wps[:, 0:P], ones_r[:], ones_r[:, 0:P],
                                 start=True, stop=True)

            # ==== software-pipelined emission ====
            # PE is in-order: feed the activation engine (the critical
            # resource) as early and as continuously as possible.
            # Q projection pairs share wide psum tiles so the conversion
            # chain is not serialized behind the "ps" ring.
            for mp in [0]:
                psq = ps_s.tile([P, KTG * QB], f32, tag="s", name="psq")
                for mh in range(2):
                    m = 2 * mp + mh
                    proj_dr(psq[:, mh * QB:(mh + 1) * QB],
                            sb["w18"], sb["w1r"], sb["xq8"], sb["xqr"],
                            slice(m * P, (m + 1) * P), slice(None))
                for mh in range(2):
                    m = 2 * mp + mh
                    for hp in range(2):
                        h = 2 * m + hp
                        base = hp * HD
                        nc.vector.tensor_scalar(
                            QM[h // 4][base:base + HD,
                                       (NKT + h % 4) * QB:
                                       (NKT + h % 4 + 1) * QB],
                            psq[base:base + HD, mh * QB:(mh + 1) * QB],
                            b1sb[base:base + HD, m:m + 1], None, Alu.add)
            emit_k_proj(0)
            for mp in [1]:
                psq = ps_s.tile([P, KTG * QB], f32, tag="s", name="psq")
                for mh in range(2):
                    m = 2 * mp + mh
                    proj_dr(psq[:, mh * QB:(mh + 1) * QB],
                            sb["w18"], sb["w1r"], sb["xq8"], sb["xqr"],
                            slice(m * P, (m + 1) * P), slice(None))
                for mh in range(2):
                    m = 2 * mp + mh
                    for hp in range(2):
                        h = 2 * m + hp
                        base = hp * HD
                        nc.vector.tensor_scalar(
                            QM[h // 4][base:base + HD,
                                       (NKT + h % 4) * QB:
                                       (NKT + h % 4 + 1) * QB],
                            psq[base:base + HD, mh * QB:(mh + 1) * QB],
                            b1sb[base:base + HD, m:m + 1], None, Alu.add)
            sc = {0: emit_scores_exp(0)}
            emit_k_proj(1)
            sc[1] = emit_scores_exp(1)
            emit_k_proj(2)
            sc[2] = emit_scores_exp(2)
            # broadcast bo across partitions via a K=1 matmul (emitted here
            # so its late DMA never gates the Q/K projections at the head of
            # the in-order PE stream)
            psb2 = ps_a.tile([P, D], f32, tag="pv1", name="psb2", bufs=1)
            nc.tensor.matmul(psb2[:], ones_r[:], borow[:], start=True,
                             stop=True)
            nc.vector.tensor_copy(bob[:], psb2[:])
            emit_v_proj(0, 8)
            emit_k_proj(3)
            sc[3] = emit_scores_exp(3)
            emit_v_proj(8, 16)
            emit_pv(0, sc.pop(0))
            sc[4] = emit_scores_exp(4)
            emit_pv(1, sc.pop(1))
            sc[5] = emit_scores_exp(5)
            emit_pv(2, sc.pop(2))
            sc[6] = emit_scores_exp(6)
            emit_pv(3, sc.pop(3))
            sc[7] = emit_scores_exp(7)
            emit_pv(4, sc.pop(4))
            emit_pv(5, sc.pop(5))
            emit_transpose(0)
            emit_transpose(1)
            emit_pv(6, sc.pop(6))
            emit_transpose(2)
            for qt in range(NQB):
                emit_tail_early(qt)
            emit_pv(7, sc.pop(7), t_outer=True)
            for qt in range(NQB):
                pst = ps_a.tile([P, P], bf16, tag=f"pv{qt % 2}",
                                name="pst3", bufs=1)
                nc.tensor.matmul(
                    pst[:], out2[qt][:, (NDC - 1) * P:], sb["idb_d"][:],
                    start=True, stop=True, is_transpose=True)
                nc.vector.tensor_copy(
                    oTT[:, NDC - 1, qt * P:(qt + 1) * P], pst[:])
                emit_tail_late(qt)

    nc.finalize()
    return nc


_CACHE = {}


def _get_nc():
    if "nc" not in _CACHE:
        _CACHE["nc"] = _build_nc()
    return _CACHE["nc"]


F8NP = cdt.np(cdt.float8e4)
BF16NP = cdt.np(cdt.bfloat16)


def _to_chunked(a, inner):
    """[D, N] -> [P, NDC, N] with row d = c*P+p mapped to [p, c, :]."""
    return np.ascontiguousarray(
        a.reshape(NDC, P, inner).transpose(1, 0, 2))


def _split8(a):
    """Double-fp8 decomposition: a ~ hi + lo with both parts fp8e4."""
    hi = a.astype(F8NP)
    lo = (a - hi.astype(np.float32)).astype(F8NP)
    return hi, lo


def _prep_shared(W1, b1, W2, b2, W3, b3, Wo, bo):
    f = np.float32
    id8 = np.zeros((P, P), f)
    np.fill_diagonal(id8, IDENTV)
    idb = np.zeros((P, P), f)
    np.fill_diagonal(idb, 1.0)
    shared = {
        "id8_d": id8.astype(F8NP),
        "idb_d": idb.astype(BF16NP),
        "wo_d": _to_chunked(np.ascontiguousarray(np.asarray(Wo, f).T),
                            D).astype(BF16NP),
        "b1_d": np.ascontiguousarray(
            (np.asarray(b1, f) * f(WS)).reshape(NDC, P).T),
        "b2_d": np.ascontiguousarray(
            (np.asarray(b2, f) * f(WS)).reshape(NDC, P).T),
        # v-bias folds through the attention average and the output
        # projection exactly: y = (attn + b3) @ Wo.T + bo
        "bo_d": (np.asarray(bo, f)
                 + np.asarray(b3, f) @ np.asarray(Wo, f).T
                 ).reshape(1, D).copy(),
    }
    for nm, W in [("w1", W1), ("w2", W2), ("w3", W3)]:
        hi, lo = _split8(_to_chunked(np.asarray(W, f).T * f(WS), D))
        shared[nm + "8"] = hi
        shared[nm + "r"] = lo
    return shared


def build_in_maps(q_in, k_in, v_in, mask, W1, b1, W2, b2, W3, b3, Wo, bo):
    f = np.float32
    q_in = np.asarray(q_in, f)
    k_in = np.asarray(k_in, f)
    v_in = np.asarray(v_in, f)
    mask = np.asarray(mask)
    shared = _prep_shared(W1, b1, W2, b2, W3, b3, Wo, bo)
    kc = [_split8(_to_chunked(k_in[b].T, S)) for b in range(B)]
    kc = [(np.ascontiguousarray(h[:, :, :QB]),
           np.ascontiguousarray(h[:, :, QB:]),
           np.ascontiguousarray(l[:, :, :QB]),
           np.ascontiguousarray(l[:, :, QB:])) for h, l in kc]
    vc = [_split8(_to_chunked(v_in[b].T, S)) for b in range(B)]
    in_maps = []
    for c in range(NCORES):
        b, r = divmod(c, RPB)
        sl = slice(r * QB, (r + 1) * QB)
        # masked (m==0) entries get MASKV (=-120), unmasked get 0
        mt = (f(1.0) - mask[b, 0, sl, :].T.astype(f)) * f(MASKV)
        q8, qr = _split8(_to_chunked(
            np.ascontiguousarray(q_in[b, sl, :].T), QB))
        in_maps.append({
            "xq8": q8, "xqr": qr,
            "xk8a": kc[b][0], "xk8b": kc[b][1],
            "xkra": kc[b][2], "xkrb": kc[b][3],
            "xv8": vc[b][0], "xvr": vc[b][1],
            "m_d": np.ascontiguousarray(
                mt.reshape(NKT, P, QB)[::-1].transpose(1, 0, 2)).astype(F8NP),
            **shared,
        })
    return in_maps


def kernel(q_in, k_in, v_in, mask, W1, b1, W2, b2, W3, b3, Wo, bo):
    f = np.float32
    nc = _get_nc()
    in_maps = build_in_maps(q_in, k_in, v_in, mask, W1, b1, W2, b2, W3, b3,
                            Wo, bo)
    res = run_bass_kernel_spmd(nc, in_maps, list(range(NCORES)))
    out = np.empty((B, S, D), f)
    for c in range(NCORES):
        b, r = divmod(c, RPB)
        out[b, r * QB:(r + 1) * QB, :] = res.results[c]["y"]
    return out



# revision 35
# speedup vs baseline: 1.1113x; 1.0601x over previous
"""Multi-head attention on 8 Trainium2 NeuronCores — fp8 DoubleRow version.

Sharding: batch (2) x query-row-block (4) -> 8 cores; each core computes full
attention for its 512 query rows of one batch (K/V projected for all keys).

Techniques vs the fp32r baseline:
  - QKV projections run as fp8e4 DoubleRow matmuls on a host-side double-fp8
    decomposition (x ~ x8 + xr, W ~ w8 + wr, keeping the three first-order
    products) giving ~bf16 accuracy at fp8-DR speed.  Weights are scaled x8
    on host so fp8 quantization stays in e4m3 normal range.
  - Scores: one DoubleRow matmul per (head, key-tile) computes q.k AND adds
    the attention mask: slice 0 contracts the zero-padded per-head q against
    the two-head kT tile; slice 1 multiplies a 128*I identity into a {0,-120}
    mask tile.  PSUM gets qk - 15360*(1-m), i.e. s - 30*(1-m) after the 2^-9
    exp() activation scale (which also folds 1/sqrt(HD) and the x8 weight
    scales).  exp(s-30) underflows to exactly 0, matching the reference mask.
  - p@V runs with exp tiles (fp16) as the stationary operand: output lands
    as [q, 64v+1ones] per head with only 65 moving rows per instruction, and
    softmax denominators are normalized by a per-partition reciprocal via
    tensor_scalar.  A cheap PE transpose pass restores the [d, q] layout for
    the bf16 output projection.
Host side only reshapes/quantizes inputs and concatenates outputs.
"""

import numpy as np
import concourse.bass as bass
import concourse.mybir as mybir
from concourse import bacc
from concourse.dt import dt as cdt
from concourse.tile import TileContext
from concourse.bass_utils import run_bass_kernel_spmd

B, S, D, H, HD = 2, 2048, 512, 8, 64
P = 128
NCORES = 8
RPB = 4              # q-row blocks per batch
QB = S // RPB        # 512 query rows per core
NDC = D // P         # 4 chunks of the model dim
NKT = S // P         # 16 key tiles
KTG = 2              # key tiles per scores/exp group
NG = NKT // KTG      # 8 groups
VW = HD + 1          # 65 columns per head in VE (64 v + 1 ones)
NQB = QB // P        # 4 query-partition blocks

WS = 8.0             # host-side weight scale (keeps fp8 weights normal)
EXP_SCALE = 1.0 / 512.0   # 2^-9: undoes WS*WS and applies 1/sqrt(HD)
MASKV = -120.0       # mask tile value for masked-out entries
IDENTV = 128.0       # identity scale; IDENTV*MASKV*EXP_SCALE = -30
ONESV = 8.0          # V ones-column value (den = 8*sum(p); num = 8*sum(p*v))

# Schraudolph fp16 exp for the DVE/Pool engines: exp(psum/512) ~=
# bitcast_f16(int16(round(psum * 2*log2(e) + 15300))).  One tensor_scalar
# (mult, add) with an int16 output gives the rounded bits; the int16 tile is
# bitcast to f16 when consumed.  Max relative error ~3.3% on a sawtooth;
# applied to half the score tiles it costs ~1.5e-3 extra end-to-end error.
SCH_A = 2 * 1.4426950408889634
SCH_B = 15300.0

f32 = mybir.dt.float32
f32r = mybir.dt.float32r
bf16 = mybir.dt.bfloat16
f16 = mybir.dt.float16
f8 = mybir.dt.float8e4
u32 = mybir.dt.uint32
i16 = mybir.dt.int16
Alu = mybir.AluOpType
ActF = mybir.ActivationFunctionType
DR = mybir.MatmulPerfMode.DoubleRow

QM_W = (NKT + H // 2) * QB         # reversed mask tiles + 4 q slots
KI_W = P + S                       # fp8 identity + one d-chunk of kts

PHASES = []                        # (label, first_instruction_id) markers


def _mark(nc, label):
    PHASES.append((label, nc.get_next_instruction_name()))


def _build_nc():
    nc = bacc.Bacc("TRN2", target_bir_lowering=False, debug=False,
                   num_devices=NCORES)
    dram = {}
    for nm, shape, dt_ in [
        ("xq8", [P, NDC, QB], f8), ("xqr", [P, NDC, QB], f8),
        ("xk8a", [P, NDC, QB], f8), ("xk8b", [P, NDC, S - QB], f8),
        ("xkra", [P, NDC, QB], f8), ("xkrb", [P, NDC, S - QB], f8),
        ("xv8", [P, NDC, S], f8), ("xvr", [P, NDC, S], f8),
        ("m_d", [P, NKT, QB], f8),
        ("w18", [P, NDC, D], f8), ("w1r", [P, NDC, D], f8),
        ("w28", [P, NDC, D], f8), ("w2r", [P, NDC, D], f8),
        ("w38", [P, NDC, D], f8), ("w3r", [P, NDC, D], f8),
        ("wo_d", [P, NDC, D], bf16),
        ("id8_d", [P, P], f8), ("idb_d", [P, P], bf16),
        ("b1_d", [P, NDC], f32), ("b2_d", [P, NDC], f32),
        ("bo_d", [1, D], f32),
    ]:
        dram[nm] = nc.dram_tensor(nm, shape, dt_, kind="ExternalInput")
    y = nc.dram_tensor("y", [QB, D], f32, kind="ExternalOutput")

    with TileContext(nc) as tc, nc.allow_low_precision("fp8 attention"):
        with (
            tc.tile_pool(name="persist", bufs=1) as pp,
            tc.tile_pool(name="small", bufs=1) as sp,
            tc.tile_pool(name="ps_s", bufs=3, space="PSUM") as ps_s,
            tc.tile_pool(name="ps_a", bufs=2, space="PSUM") as ps_a,
            tc.tile_pool(name="pex", bufs=34) as pex,
            tc.tile_pool(name="psx", bufs=5) as psx,
            tc.tile_pool(name="yp", bufs=2) as yp,
        ):
            # ---- persistent SBUF tiles ----
            KI = [pp.tile([P, KI_W], f8, tag=f"KI{m}", name=f"KI{m}")
                  for m in range(1, NDC)]
            KI0 = [pp.tile([P, P + QB], f8, tag=f"KI0_{kb}",
                           name=f"KI0_{kb}") for kb in range(S // QB)]
            QM = [pp.tile([P, QM_W], f8, tag=f"QM{i}", name=f"QM{i}")
                  for i in range(2)]
            VE = pp.tile([P, NKT * H * VW], f16, tag="VE", name="VE")
            sb = {}
            for nm, w, dt_ in [
                ("xq8", NDC * QB, f8), ("xqr", NDC * QB, f8),
                ("xk8a", NDC * QB, f8), ("xk8b", NDC * (S - QB), f8),
                ("xkra", NDC * QB, f8), ("xkrb", NDC * (S - QB), f8),
                ("xv8", NDC * S, f8), ("xvr", NDC * S, f8),
                ("w18", NDC * D, f8), ("w1r", NDC * D, f8),
                ("w28", NDC * D, f8), ("w2r", NDC * D, f8),
                ("w38", NDC * D, f8), ("w3r", NDC * D, f8),
                ("wo_d", NDC * D, bf16), ("idb_d", P, bf16),
            ]:
                inner = w // (NDC if nm not in ("idb_d",) else 1)
                if nm == "idb_d":
                    sb[nm] = pp.tile([P, P], dt_, tag=nm, name=nm)
                else:
                    sb[nm] = pp.tile([P, NDC, inner], dt_, tag=nm, name=nm)
            out2 = [pp.tile([P, D], bf16, tag=f"o2_{qb}", name=f"o2_{qb}")
                    for qb in range(NQB)]
            outT = [pp.tile([P, QB], bf16, tag=f"oT{m}", name=f"oT{m}")
                    for m in range(NDC)]

            b1sb = sp.tile([P, NDC], f32, tag="b1sb", name="b1sb")
            b2sb = sp.tile([P, NDC], f32, tag="b2sb", name="b2sb")
            borow = sp.tile([1, D], f32r, tag="borow", name="borow")
            bob = sp.tile([P, D], f32, tag="bob", name="bob")
            ones_r = sp.tile([1, P], f32r, tag="ones_r", name="ones_r")
            nc.vector._memset_packed(ones_r[:].bitcast(u32), 0x3F800000)

            # bulk DMAs split across SP and Pool rings; Q/K/mask inputs
            # first so the attention pipeline can start early
            for nm in ["xq8", "w18", "xqr"]:
                nc.sync.dma_start(sb[nm][:], dram[nm][:])
            nc.sync.dma_start(sb["xk8a"][:], dram["xk8a"][:])
            nc.sync.dma_start(sb["w28"][:], dram["w28"][:])
            nc.sync.dma_start(b1sb[:], dram["b1_d"][:])
            nc.sync.dma_start(sb["xk8b"][:], dram["xk8b"][:])
            for kb in range(S // QB):
                nc.sync.dma_start(KI0[kb][:, 0:P], dram["id8_d"][:])
            for m in range(NDC - 1):
                nc.sync.dma_start(KI[m][:, 0:P], dram["id8_d"][:])
            nc.sync.dma_start(sb["wo_d"][:], dram["wo_d"][:])
            nc.sync.dma_start(borow[:], dram["bo_d"][:].squeeze().bitcast(f32r))
            nc.sync.dma_start(sb["idb_d"][:], dram["idb_d"][:])
            nc.gpsimd.dma_start(
                QM[0][:, (NKT // 2) * QB:NKT * QB].rearrange(
                    "p (t q) -> p t q", t=NKT // 2),
                dram["m_d"][:, NKT // 2:, :])
            nc.gpsimd.dma_start(sb["w1r"][:], dram["w1r"][:])
            nc.gpsimd.dma_start(sb["xkra"][:], dram["xkra"][:])
            nc.gpsimd.dma_start(sb["w2r"][:], dram["w2r"][:])
            nc.gpsimd.dma_start(b2sb[:], dram["b2_d"][:])
            nc.gpsimd.dma_start(
                QM[0][:, 0:(NKT // 2) * QB].rearrange(
                    "p (t q) -> p t q", t=NKT // 2),
                dram["m_d"][:, 0:NKT // 2, :])
            nc.sync.dma_start(
                QM[1][:, 0:NKT * QB].rearrange(
                    "p (t q) -> p t q", t=NKT), dram["m_d"][:])
            for nm in ["xkrb", "xv8", "w38", "xvr", "w3r"]:
                nc.gpsimd.dma_start(sb[nm][:], dram[nm][:])

            # zero the per-head q slots (conversions fill 64 rows per slot)
            qmz = QM[0][:, NKT * QB:].bitcast(u32)
            nc.scalar.mul(qmz, qmz, 0.0)
            qmz1 = QM[1][:, NKT * QB:].bitcast(u32)
            nc.vector._memset_packed(qmz1, 0)
            # V ones-columns
            VEv = VE[:].rearrange("p (t h c) -> p t h c", t=NKT, c=VW)
            nc.gpsimd.memset(VEv[:, :, :, HD:VW], ONESV)

            def proj_dr(ps_ap, wmain, wres, xmain, xres, mcols, xcols):
                """psum += (w8+wr).T x8 + w8.T xr over 4 chunks, 3 DR mms."""
                pairs = [(wmain, xmain), (wmain, xres), (wres, xmain)]
                for i, (wt, xt) in enumerate(pairs):
                    for j in range(NDC // 2):
                        nc.tensor.matmul(
                            ps_ap,
                            wt[:, 2 * j:2 * j + 2, mcols],
                            xt[:, 2 * j:2 * j + 2, xcols],
                            start=(i == 0 and j == 0),
                            stop=(i == 2 and j == NDC // 2 - 1),
                            perf_mode=DR)

            def emit_q_proj(m):
                # Q projection chunk -> zero-padded per-head fp8 slots in QM
                ps = psp.tile([P, QB], f32, tag="ps", name="psq")
                proj_dr(ps[:], sb["w18"], sb["w1r"], sb["xq8"], sb["xqr"],
                        slice(m * P, (m + 1) * P), slice(None))
                for hp in range(2):
                    h = 2 * m + hp
                    base = hp * HD
                    nc.vector.tensor_scalar(
                        QM[base:base + HD,
                           (NKT + h) * QB:(NKT + h + 1) * QB],
                        ps[base:base + HD, :],
                        b1sb[base:base + HD, m:m + 1], None, Alu.add)

            def emit_k_proj(m):
                # K projection chunk -> fp8 kts slot m in KI.  The m=0
                # conversions run on the (otherwise idle) scalar engine so
                # the startup isn't serialized behind the DVE queue.
                for kb in range(S // QB):
                    ps = ps_a.tile([P, QB], f32, tag=f"pv{kb % 2}",
                                   name="psk", bufs=1)
                    if kb == 0:
                        proj_dr(ps[:], sb["w28"], sb["w2r"],
                                sb["xk8a"], sb["xkra"],
                                slice(m * P, (m + 1) * P), slice(None))
                    else:
                        proj_dr(ps[:], sb["w28"], sb["w2r"],
                                sb["xk8b"], sb["xkrb"],
                                slice(m * P, (m + 1) * P),
                                slice((kb - 1) * QB, kb * QB))
                    if m == 0:
                        dst = KI0[kb][:, P:]
                        if kb == 0:
                            nc.scalar.add(dst, ps[:], b2sb[:, m:m + 1])
                        else:
                            nc.vector.tensor_scalar(
                                dst, ps[:], b2sb[:, m:m + 1], None, Alu.add)
                    elif m == 1:
                        dst = KI[m - 1][:, P + kb * QB: P + (kb + 1) * QB]
                        nc.vector.tensor_scalar(
                            dst, ps[:], b2sb[:, m:m + 1], None, Alu.add)
                    else:
                        dst = KI[m - 1][:, P + kb * QB: P + (kb + 1) * QB]
                        nc.scalar.add(dst, ps[:], b2sb[:, m:m + 1])

            VEv = VE[:].rearrange("p (t h c) -> p t h c", t=NKT, c=VW)

            def emit_v_proj(t0, t1):
                # V projection tiles -> fp16 VE (head-interleaved; the v bias
                # b3 is folded into the output-projection bias on the host,
                # so this is a pure convert, split across ACT and DVE)
                for t in range(t0, t1):
                    ps = ps_a.tile([P, D], f32, tag=f"pv{t % 2}",
                                   name="psv", bufs=1)
                    pairs = [(sb["xv8"], sb["w38"]), (sb["xvr"], sb["w38"]),
                             (sb["xv8"], sb["w3r"])]
                    for i, (xt, wt) in enumerate(pairs):
                        for j in range(NDC // 2):
                            nc.tensor.matmul(
                                ps[:],
                                xt[:, 2 * j:2 * j + 2, t * P:(t + 1) * P],
                                wt[:, 2 * j:2 * j + 2, :],
                                start=(i == 0 and j == 0),
                                stop=(i == 2 and j == NDC // 2 - 1),
                                perf_mode=DR)
                    src = ps[:].rearrange("p (h d) -> p h d", d=HD)
                    if t % 2 == 0:
                        nc.scalar.copy(VEv[:, t, :, 0:HD], src)
                    else:
                        nc.vector.tensor_copy(VEv[:, t, :, 0:HD], src)

            # ==== attention building blocks ====
            KIv = [KI[m][:].rearrange("p (s c) -> p s c", c=P)
                   for m in range(NDC - 1)]
            KI0v = [KI0[kb][:].rearrange("p (s c) -> p s c", c=P)
                    for kb in range(S // QB)]
            QMv = [QM[i][:].rearrange("p (s q) -> p s q", q=QB)
                   for i in range(2)]

            # exp engine per (head, group): A = exact exp on ACT; V =
            # Schraudolph int16 trick on DVE; P = two-stage lane for the
            # PSUM-blind Pool engine (ACT/DVE copies the f32 scores to an
            # f16 staging tile, Pool runs the Schraudolph from SBUF).  The
            # DVE/Pool share ramps up as projection/DMA duties wind down.
            EXP_ENG = {0: "AAVAAVAA", 1: "AVAAVAAV", 2: "AVAVAAVA",
                       4: "AVAVAVAA", 6: "AVAVAVAA"}
            EXP_ENG_LATE = "AVAVAVAV"

            def emit_scores_exp(h):
                m = h // 2
                engs = EXP_ENG.get(h, EXP_ENG_LATE)
                pes = []
                for g in range(NG):
                    sg = ps_s.tile([P, KTG * QB], f32, tag="s", name="s")
                    for j in range(KTG):
                        t = g * KTG + j
                        # slice 0: (identity, reversed-mask tile)
                        # slice 1: (kT tile, zero-padded per-head q)
                        if m == 0:
                            kv = KI0v[t // 4]
                            ks = 1 + t % 4
                        else:
                            kv = KIv[m - 1]
                            ks = 1 + t
                        ms = NKT - 1 - t
                        dq = NKT + h % 4 - ms
                        nc.tensor.matmul(
                            sg[:, j * QB:(j + 1) * QB],
                            kv[:, 0:ks + 1:ks, :],
                            QMv[h // 4][:, ms:ms + dq + 1:dq, :],
                            start=True, stop=True, perf_mode=DR)
                    if engs[g] == "A":
                        pe = pex.tile([P, KTG * QB], f16, tag="pe", name="pe")
                        nc.scalar.activation(pe[:], sg[:], ActF.Exp,
                                             scale=EXP_SCALE)
                        pes.append(pe[:])
                    elif engs[g] == "V":
                        pe = pex.tile([P, KTG * QB], i16, tag="pe", name="pe")
                        nc.vector.tensor_scalar(pe[:], sg[:], SCH_A, SCH_B,
                                                Alu.mult, op1=Alu.add)
                        pes.append(pe[:].bitcast(f16))
                    else:
                        st = psx.tile([P, KTG * QB], f16, tag="st", name="st")
                        if g % 2 == 0:
                            nc.scalar.copy(st[:], sg[:])
                        else:
                            nc.vector.tensor_copy(st[:], sg[:])
                        pe = pex.tile([P, KTG * QB], i16, tag="pe", name="pe")
                        nc.gpsimd.tensor_scalar(pe[:], st[:], SCH_A, SCH_B,
                                                Alu.mult, op1=Alu.add)
                        pes.append(pe[:].bitcast(f16))
                return pes

            def emit_pv(h, pes, t_outer=False):
                # p @ [V | ones]: p tiles stationary -> [q, 65] outputs
                if t_outer:
                    # last head: qb2/qb3 ride retired score-pool banks and
                    # pre-accumulate t<15; qb0/qb1 interleave by t; only
                    # four t=15 matmuls depend on the final exp
                    pvs = [ps_a.tile([P, VW], f32, tag=f"pv{qb}",
                                     name="pv", bufs=1) for qb in range(2)]
                    pvs += [ps_s.tile([P, VW], f32, tag="s", name="pvs")
                            for _ in range(2)]
                    for qb in (2, 3):
                        for t in range(NKT - 1):
                            g, j = divmod(t, KTG)
                            nc.tensor.matmul(
                                pvs[qb][:],
                                pes[g][:, j * QB + qb * P:
                                       j * QB + (qb + 1) * P],
                                VEv[:, t, h, :],
                                start=(t == 0), stop=False)
                    for t in range(NKT):
                        g, j = divmod(t, KTG)
                        for qb in range(2):
                            nc.tensor.matmul(
                                pvs[qb][:],
                                pes[g][:, j * QB + qb * P:
                                       j * QB + (qb + 1) * P],
                                VEv[:, t, h, :],
                                start=(t == 0), stop=(t == NKT - 1))
                    t, (g, j) = NKT - 1, divmod(NKT - 1, KTG)
                    for qb in (2, 3):
                        nc.tensor.matmul(
                            pvs[qb][:],
                            pes[g][:, j * QB + qb * P:
                                   j * QB + (qb + 1) * P],
                            VEv[:, t, h, :],
                            start=False, stop=True)
                for qb in range(NQB):
                    if t_outer:
                        pv = pvs[qb]
                    else:
                        pv = ps_a.tile([P, VW], f32, tag=f"pv{qb % 2}",
                                       name="pv", bufs=1)
                        for t in range(NKT):
                            g, j = divmod(t, KTG)
                            nc.tensor.matmul(
                                pv[:],
                                pes[g][:, j * QB + qb * P:
                                       j * QB + (qb + 1) * P],
                                VEv[:, t, h, :],
                                start=(t == 0), stop=(t == NKT - 1))
                    rcp = yp.tile([P, 1], f32, tag="rcp", name="rcp", bufs=2)
                    nc.vector.reciprocal(rcp[:], pv[:, HD:VW])
                    nc.vector.tensor_scalar(
                        out2[qb][:, h * HD:(h + 1) * HD],
                        pv[:, 0:HD], rcp[:], None, Alu.mult)

            oTT = pp.tile([P, NDC, QB], bf16, tag="oTT", name="oTT")

            def emit_transpose(m):
                # transpose out2[:, d-chunk m] -> oTT[:, m, :] ([d, q] layout)
                pst = ps_a.tile([P, QB], bf16, tag=f"pv{m % 2}",
                                name="pst", bufs=1)
                for qt in range(NQB):
                    nc.tensor.matmul(
                        pst[:, qt * P:(qt + 1) * P],
                        out2[qt][:, m * P:(m + 1) * P],
                        sb["idb_d"][:],
                        start=True, stop=True, is_transpose=True)
                nc.vector.tensor_copy(oTT[:, m, :], pst[:])

            ypb = [pp.tile([P, D], f32, tag=f"ypb{qt}", name=f"ypb{qt}")
                   for qt in range(NQB)]

            def emit_tail_early(qt):
                # partial output projection over d-chunks 0..2 (+ bias)
                ps = ps_a.tile([P, D], f32, tag=f"pv{qt % 2}",
                               name="psy", bufs=1)
                for m in range(NDC - 1):
                    nc.tensor.matmul(
                        ps[:], oTT[:, m, qt * P:(qt + 1) * P],
                        sb["wo_d"][:, m, :],
                        start=(m == 0), stop=(m == NDC - 2))
                nc.vector.tensor_tensor(ypb[qt][:], ps[:], bob[:],
                                        op=Alu.add)

            def emit_tail_late(qt):
                # last d-chunk + partial sum + store for one q block; the
                # y DMAs ride the SP and (idle by now) ACT hwdge queues
                ps = ps_a.tile([P, D], f32, tag=f"pv{qt % 2}",
                               name="psy", bufs=1)
                nc.tensor.matmul(
                    ps[:], oTT[:, NDC - 1, qt * P:(qt + 1) * P],
                    sb["wo_d"][:, NDC - 1, :], start=True, stop=True)
                ysb = yp.tile([P, D], f32, tag="ysb", name="ysb", bufs=2)
                nc.vector.tensor_tensor(ysb[:], ps[:], ypb[qt][:],
                                        op=Alu.add)
                eng = nc.sync if qt % 2 == 0 else nc.scalar
                eng.dma_start(y[qt * P:(qt + 1) * P, :], ysb[:])

            # ==== PE p-state warm-up ====
            wps = ps_a.tile([P, D], f32, tag="pv0", name="wps", bufs=1)
            for w in range(15):
                nc.tensor.matmul(wps[:, 0:P], ones_r[:], ones_r[:, 0:P],
                                 start=True, stop=True)

            # ==== software-pipelined emission ====
            # PE is in-order: feed the activation engine (the critical
            # resource) as early and as continuously as possible.
            # Q projection pairs share wide psum tiles so the conversion
            # chain is not serialized behind the "ps" ring.
            for mp in [0]:
                psq = ps_s.tile([P, KTG * QB], f32, tag="s", name="psq")
                for mh in range(2):
                    m = 2 * mp + mh
                    proj_dr(psq[:, mh * QB:(mh + 1) * QB],
                            sb["w18"], sb["w1r"], sb["xq8"], sb["xqr"],
                            slice(m * P, (m + 1) * P), slice(None))
                for mh in range(2):
                    m = 2 * mp + mh
                    for hp in range(2):
                        h = 2 * m + hp
                        base = hp * HD
                        nc.vector.tensor_scalar(
                            QM[h // 4][base:base + HD,
                                       (NKT + h % 4) * QB:
                                       (NKT + h % 4 + 1) * QB],
                            psq[base:base + HD, mh * QB:(mh + 1) * QB],
                            b1sb[base:base + HD, m:m + 1], None, Alu.add)
            emit_k_proj(0)
            for mp in [1]:
                psq = ps_s.tile([P, KTG * QB], f32, tag="s", name="psq")
                for mh in range(2):
                    m = 2 * mp + mh
                    proj_dr(psq[:, mh * QB:(mh + 1) * QB],
                            sb["w18"], sb["w1r"], sb["xq8"], sb["xqr"],
                            slice(m * P, (m + 1) * P), slice(None))
                for mh in range(2):
                    m = 2 * mp + mh
                    for hp in range(2):
                        h = 2 * m + hp
                        base = hp * HD
                        nc.vector.tensor_scalar(
                            QM[h // 4][base:base + HD,
                                       (NKT + h % 4) * QB:
                                       (NKT + h % 4 + 1) * QB],
                            psq[base:base + HD, mh * QB:(mh + 1) * QB],
                            b1sb[base:base + HD, m:m + 1], None, Alu.add)
            _mark(nc, 'sc0')
            sc = {0: emit_scores_exp(0)}
            _mark(nc, 'K1')
            emit_k_proj(1)
            _mark(nc, 'sc1')
            sc[1] = emit_scores_exp(1)
            _mark(nc, 'K2')
            emit_k_proj(2)
            _mark(nc, 'sc2')
            sc[2] = emit_scores_exp(2)
            # broadcast bo across partitions via a K=1 matmul (emitted here
            # so its late DMA never gates the Q/K projections at the head of
            # the in-order PE stream)
            psb2 = ps_a.tile([P, D], f32, tag="pv1", name="psb2", bufs=1)
            nc.tensor.matmul(psb2[:], ones_r[:], borow[:], start=True,
                             stop=True)
            nc.vector.tensor_copy(bob[:], psb2[:])
            _mark(nc, 'V0-8')
            emit_v_proj(0, 8)
            _mark(nc, 'K3')
            emit_k_proj(3)
            _mark(nc, 'sc3')
            sc[3] = emit_scores_exp(3)
            _mark(nc, 'V8-16')
            emit_v_proj(8, 16)
            _mark(nc, 'pv0')
            emit_pv(0, sc.pop(0))
            _mark(nc, 'sc4')
            sc[4] = emit_scores_exp(4)
            _mark(nc, 'pv1')
            emit_pv(1, sc.pop(1))
            _mark(nc, 'sc5')
            sc[5] = emit_scores_exp(5)
            _mark(nc, 'pv2')
            emit_pv(2, sc.pop(2))
            _mark(nc, 'sc6')
            sc[6] = emit_scores_exp(6)
            _mark(nc, 'pv3')
            emit_pv(3, sc.pop(3))
            _mark(nc, 'sc7')
            sc[7] = emit_scores_exp(7)
            _mark(nc, 'pv4')
            emit_pv(4, sc.pop(4))
            _mark(nc, 'pv5')
            emit_pv(5, sc.pop(5))
            _mark(nc, 'T0')
            emit_transpose(0)
            emit_transpose(1)
            _mark(nc, 'pv6')
            emit_pv(6, sc.pop(6))
            emit_transpose(2)
            _mark(nc, 'tailE')
            for qt in range(NQB):
                emit_tail_early(qt)
            _mark(nc, 'pv7')
            emit_pv(7, sc.pop(7), t_outer=True)
            _mark(nc, 'tailL')
            for qt in range(NQB):
                pst = ps_a.tile([P, P], bf16, tag=f"pv{qt % 2}",
                                name="pst3", bufs=1)
                nc.tensor.matmul(
                    pst[:], out2[qt][:, (NDC - 1) * P:], sb["idb_d"][:],
                    start=True, stop=True, is_transpose=True)
                nc.vector.tensor_copy(
                    oTT[:, NDC - 1, qt * P:(qt + 1) * P], pst[:])
                emit_tail_late(qt)

    nc.finalize()
    return nc


_CACHE = {}


def _get_nc():
    if "nc" not in _CACHE:
        _CACHE["nc"] = _build_nc()
    return _CACHE["nc"]


F8NP = cdt.np(cdt.float8e4)
BF16NP = cdt.np(cdt.bfloat16)


def _to_chunked(a, inner):
    """[D, N] -> [P, NDC, N] with row d = c*P+p mapped to [p, c, :]."""
    return np.ascontiguousarray(
        a.reshape(NDC, P, inner).transpose(1, 0, 2))


def _split8(a):
    """Double-fp8 decomposition: a ~ hi + lo with both parts fp8e4."""
    hi = a.astype(F8NP)
    lo = (a - hi.astype(np.float32)).astype(F8NP)
    return hi, lo


def _prep_shared(W1, b1, W2, b2, W3, b3, Wo, bo):
    f = np.float32
    id8 = np.zeros((P, P), f)
    np.fill_diagonal(id8, IDENTV)
    idb = np.zeros((P, P), f)
    np.fill_diagonal(idb, 1.0)
    shared = {
        "id8_d": id8.astype(F8NP),
        "idb_d": idb.astype(BF16NP),
        "wo_d": _to_chunked(np.ascontiguousarray(np.asarray(Wo, f).T),
                            D).astype(BF16NP),
        "b1_d": np.ascontiguousarray(
            (np.asarray(b1, f) * f(WS)).reshape(NDC, P).T),
        "b2_d": np.ascontiguousarray(
            (np.asarray(b2, f) * f(WS)).reshape(NDC, P).T),
        # v-bias folds through the attention average and the output
        # projection exactly: y = (attn + b3) @ Wo.T + bo
        "bo_d": (np.asarray(bo, f)
                 + np.asarray(b3, f) @ np.asarray(Wo, f).T
                 ).reshape(1, D).copy(),
    }
    for nm, W in [("w1", W1), ("w2", W2), ("w3", W3)]:
        hi, lo = _split8(_to_chunked(np.asarray(W, f).T * f(WS), D))
        shared[nm + "8"] = hi
        shared[nm + "r"] = lo
    return shared


def build_in_maps(q_in, k_in, v_in, mask, W1, b1, W2, b2, W3, b3, Wo, bo):
    f = np.float32
    q_in = np.asarray(q_in, f)
    k_in = np.asarray(k_in, f)
    v_in = np.asarray(v_in, f)
    mask = np.asarray(mask)
    shared = _prep_shared(W1, b1, W2, b2, W3, b3, Wo, bo)
    kc = [_split8(_to_chunked(k_in[b].T, S)) for b in range(B)]
    kc = [(np.ascontiguousarray(h[:, :, :QB]),
           np.ascontiguousarray(h[:, :, QB:]),
           np.ascontiguousarray(l[:, :, :QB]),
           np.ascontiguousarray(l[:, :, QB:])) for h, l in kc]
    vc = [_split8(_to_chunked(v_in[b].T, S)) for b in range(B)]
    in_maps = []
    for c in range(NCORES):
        b, r = divmod(c, RPB)
        sl = slice(r * QB, (r + 1) * QB)
        # masked (m==0) entries get MASKV (=-120), unmasked get 0
        mt = (f(1.0) - mask[b, 0, sl, :].T.astype(f)) * f(MASKV)
        q8, qr = _split8(_to_chunked(
            np.ascontiguousarray(q_in[b, sl, :].T), QB))
        in_maps.append({
            "xq8": q8, "xqr": qr,
            "xk8a": kc[b][0], "xk8b": kc[b][1],
            "xkra": kc[b][2], "xkrb": kc[b][3],
            "xv8": vc[b][0], "xvr": vc[b][1],
            "m_d": np.ascontiguousarray(
                mt.reshape(NKT, P, QB)[::-1].transpose(1, 0, 2)).astype(F8NP),
            **shared,
        })
    return in_maps


def kernel(q_in, k_in, v_in, mask, W1, b1, W2, b2, W3, b3, Wo, bo):
    f = np.float32
    nc = _get_nc()
    in_maps = build_in_maps(q_in, k_in, v_in, mask, W1, b1, W2, b2, W3, b3,
                            Wo, bo)
    res = run_bass_kernel_spmd(nc, in_maps, list(range(NCORES)))
    out = np.empty((B, S, D), f)
    for c in range(NCORES):
        b, r = divmod(c, RPB)
        out[b, r * QB:(r + 1) * QB, :] = res.results[c]["y"]
    return out



# revision 37
# speedup vs baseline: 1.1869x; 1.0681x over previous
"""Multi-head attention on 8 Trainium2 NeuronCores — fp8 DoubleRow version.

Sharding: batch (2) x query-row-block (4) -> 8 cores; each core computes full
attention for its 512 query rows of one batch (K/V projected for all keys).

Techniques vs the fp32r baseline:
  - QKV projections run as fp8e4 DoubleRow matmuls on a host-side double-fp8
    decomposition (x ~ x8 + xr, W ~ w8 + wr, keeping the three first-order
    products) giving ~bf16 accuracy at fp8-DR speed.  Weights are scaled x8
    on host so fp8 quantization stays in e4m3 normal range.
  - Scores: one DoubleRow matmul per (head, key-tile) computes q.k AND adds
    the attention mask: slice 0 contracts the zero-padded per-head q against
    the two-head kT tile; slice 1 multiplies a 128*I identity into a {0,-120}
    mask tile.  PSUM gets qk - 15360*(1-m), i.e. s - 30*(1-m) after the 2^-9
    exp() activation scale (which also folds 1/sqrt(HD) and the x8 weight
    scales).  exp(s-30) underflows to exactly 0, matching the reference mask.
  - p@V runs with exp tiles (fp16) as the stationary operand: output lands
    as [q, 64v+1ones] per head with only 65 moving rows per instruction, and
    softmax denominators are normalized by a per-partition reciprocal via
    tensor_scalar.  A cheap PE transpose pass restores the [d, q] layout for
    the bf16 output projection.
Host side only reshapes/quantizes inputs and concatenates outputs.
"""

import numpy as np
import concourse.bass as bass
import concourse.mybir as mybir
from concourse import bacc
from concourse.dt import dt as cdt
from concourse.tile import TileContext
from concourse.bass_utils import run_bass_kernel_spmd

B, S, D, H, HD = 2, 2048, 512, 8, 64
P = 128
NCORES = 8
RPB = 4              # q-row blocks per batch
QB = S // RPB        # 512 query rows per core
NDC = D // P         # 4 chunks of the model dim
NKT = S // P         # 16 key tiles
KTG = 2              # key tiles per scores/exp group
NG = NKT // KTG      # 8 groups
VW = HD + 1          # 65 columns per head in VE (64 v + 1 ones)
NQB = QB // P        # 4 query-partition blocks

WS = 8.0             # host-side weight scale (keeps fp8 weights normal)
EXP_SCALE = 1.0 / 512.0   # 2^-9: undoes WS*WS and applies 1/sqrt(HD)
MASKV = -120.0       # mask tile value for masked-out entries
IDENTV = 128.0       # identity scale; IDENTV*MASKV*EXP_SCALE = -30
ONESV = 8.0          # V ones-column value (den = 8*sum(p); num = 8*sum(p*v))

# Schraudolph fp16 exp for the DVE/Pool engines: exp(psum/512) ~=
# bitcast_f16(int16(round(psum * 2*log2(e) + 15300))).  One tensor_scalar
# (mult, add) with an int16 output gives the rounded bits; the int16 tile is
# bitcast to f16 when consumed.  Max relative error ~3.3% on a sawtooth;
# applied to half the score tiles it costs ~1.5e-3 extra end-to-end error.
SCH_A = 2 * 1.4426950408889634
SCH_B = 15300.0

f32 = mybir.dt.float32
f32r = mybir.dt.float32r
bf16 = mybir.dt.bfloat16
f16 = mybir.dt.float16
f8 = mybir.dt.float8e4
u32 = mybir.dt.uint32
i16 = mybir.dt.int16
Alu = mybir.AluOpType
ActF = mybir.ActivationFunctionType
DR = mybir.MatmulPerfMode.DoubleRow

QM_W = (NKT + H // 2) * QB         # reversed mask tiles + 4 q slots
KI_W = P + S                       # fp8 identity + one d-chunk of kts

PHASES = []                        # (label, first_instruction_id) markers


def _mark(nc, label):
    PHASES.append((label, nc.get_next_instruction_name()))


def _build_nc():
    nc = bacc.Bacc("TRN2", target_bir_lowering=False, debug=False,
                   num_devices=NCORES)
    dram = {}
    for nm, shape, dt_ in [
        ("xq8", [P, NDC, QB], f8), ("xqr", [P, NDC, QB], f8),
        ("xk8a", [P, NDC, QB], f8), ("xk8b", [P, NDC, S - QB], f8),
        ("xkra", [P, NDC, QB], f8), ("xkrb", [P, NDC, S - QB], f8),
        ("xv8", [P, NDC, S], f8), ("xvr", [P, NDC, S], f8),
        ("m_d", [P, NKT, QB], f8),
        ("w18", [P, NDC, D], f8), ("w1r", [P, NDC, D], f8),
        ("w28", [P, NDC, D], f8), ("w2r", [P, NDC, D], f8),
        ("w38", [P, NDC, D], f8), ("w3r", [P, NDC, D], f8),
        ("wo_d", [P, NDC, D], bf16),
        ("id8_d", [P, P], f8), ("idb_d", [P, P], bf16),
        ("b1_d", [P, NDC], f32), ("b2_d", [P, NDC], f32),
        ("bo_d", [1, D], f32),
    ]:
        dram[nm] = nc.dram_tensor(nm, shape, dt_, kind="ExternalInput")
    y = nc.dram_tensor("y", [QB, D], f32, kind="ExternalOutput")

    with TileContext(nc) as tc, nc.allow_low_precision("fp8 attention"):
        with (
            tc.tile_pool(name="persist", bufs=1) as pp,
            tc.tile_pool(name="small", bufs=1) as sp,
            tc.tile_pool(name="ps_s", bufs=2, space="PSUM") as ps_s,
            tc.tile_pool(name="ps_a", bufs=4, space="PSUM") as ps_a,
            tc.tile_pool(name="pex", bufs=28) as pex,
            tc.tile_pool(name="yp", bufs=2) as yp,
        ):
            # ---- persistent SBUF tiles ----
            KI = [pp.tile([P, KI_W], f8, tag=f"KI{m}", name=f"KI{m}")
                  for m in range(1, NDC)]
            KI0 = [pp.tile([P, P + QB], f8, tag=f"KI0_{kb}",
                           name=f"KI0_{kb}") for kb in range(S // QB)]
            QM = [pp.tile([P, QM_W], f8, tag=f"QM{i}", name=f"QM{i}")
                  for i in range(2)]
            VE = pp.tile([P, NKT * H * VW], f16, tag="VE", name="VE")
            sb = {}
            for nm, w, dt_ in [
                ("xq8", NDC * QB, f8), ("xqr", NDC * QB, f8),
                ("xk8a", NDC * QB, f8), ("xk8b", NDC * (S - QB), f8),
                ("xkra", NDC * QB, f8), ("xkrb", NDC * (S - QB), f8),
                ("xv8", NDC * S, f8), ("xvr", NDC * S, f8),
                ("w18", NDC * D, f8), ("w1r", NDC * D, f8),
                ("w28", NDC * D, f8), ("w2r", NDC * D, f8),
                ("w38", NDC * D, f8), ("w3r", NDC * D, f8),
                ("wo_d", NDC * D, bf16), ("idb_d", P, bf16),
            ]:
                inner = w // (NDC if nm not in ("idb_d",) else 1)
                if nm == "idb_d":
                    sb[nm] = pp.tile([P, P], dt_, tag=nm, name=nm)
                else:
                    sb[nm] = pp.tile([P, NDC, inner], dt_, tag=nm, name=nm)
            out2 = [pp.tile([P, D], bf16, tag=f"o2_{qb}", name=f"o2_{qb}")
                    for qb in range(NQB)]
            outT = [pp.tile([P, QB], bf16, tag=f"oT{m}", name=f"oT{m}")
                    for m in range(NDC)]

            b1sb = sp.tile([P, NDC], f32, tag="b1sb", name="b1sb")
            b2sb = sp.tile([P, NDC], f32, tag="b2sb", name="b2sb")
            borow = sp.tile([1, D], f32r, tag="borow", name="borow")
            bob = sp.tile([P, D], f32, tag="bob", name="bob")
            ones_r = sp.tile([1, P], f32r, tag="ones_r", name="ones_r")
            nc.vector._memset_packed(ones_r[:].bitcast(u32), 0x3F800000)

            # bulk DMAs split across SP and Pool rings; Q/K/mask inputs
            # first so the attention pipeline can start early
            for nm in ["xq8", "w18", "xqr"]:
                nc.sync.dma_start(sb[nm][:], dram[nm][:])
            nc.sync.dma_start(sb["xk8a"][:], dram["xk8a"][:])
            nc.sync.dma_start(sb["w28"][:], dram["w28"][:])
            nc.sync.dma_start(b1sb[:], dram["b1_d"][:])
            nc.sync.dma_start(sb["xk8b"][:], dram["xk8b"][:])
            for kb in range(S // QB):
                nc.sync.dma_start(KI0[kb][:, 0:P], dram["id8_d"][:])
            for m in range(NDC - 1):
                nc.sync.dma_start(KI[m][:, 0:P], dram["id8_d"][:])
            nc.sync.dma_start(sb["wo_d"][:], dram["wo_d"][:])
            nc.sync.dma_start(borow[:], dram["bo_d"][:].squeeze().bitcast(f32r))
            nc.sync.dma_start(sb["idb_d"][:], dram["idb_d"][:])
            nc.gpsimd.dma_start(
                QM[0][:, (NKT // 2) * QB:NKT * QB].rearrange(
                    "p (t q) -> p t q", t=NKT // 2),
                dram["m_d"][:, NKT // 2:, :])
            nc.gpsimd.dma_start(sb["w1r"][:], dram["w1r"][:])
            nc.gpsimd.dma_start(sb["xkra"][:], dram["xkra"][:])
            nc.gpsimd.dma_start(sb["w2r"][:], dram["w2r"][:])
            nc.gpsimd.dma_start(b2sb[:], dram["b2_d"][:])
            nc.gpsimd.dma_start(
                QM[0][:, 0:(NKT // 2) * QB].rearrange(
                    "p (t q) -> p t q", t=NKT // 2),
                dram["m_d"][:, 0:NKT // 2, :])
            nc.sync.dma_start(
                QM[1][:, 0:NKT * QB].rearrange(
                    "p (t q) -> p t q", t=NKT), dram["m_d"][:])
            for nm in ["xkrb", "xv8", "w38", "xvr", "w3r"]:
                nc.gpsimd.dma_start(sb[nm][:], dram[nm][:])

            # zero the per-head q slots (conversions fill 64 rows per slot)
            qmz = QM[0][:, NKT * QB:].bitcast(u32)
            nc.scalar.mul(qmz, qmz, 0.0)
            qmz1 = QM[1][:, NKT * QB:].bitcast(u32)
            nc.vector._memset_packed(qmz1, 0)
            # V ones-columns
            VEv = VE[:].rearrange("p (t h c) -> p t h c", t=NKT, c=VW)
            nc.gpsimd.memset(VEv[:, :, :, HD:VW], ONESV)

            def proj_dr(ps_ap, wmain, wres, xmain, xres, mcols, xcols):
                """psum += (w8+wr).T x8 + w8.T xr over 4 chunks, 3 DR mms."""
                pairs = [(wmain, xmain), (wmain, xres), (wres, xmain)]
                for i, (wt, xt) in enumerate(pairs):
                    for j in range(NDC // 2):
                        nc.tensor.matmul(
                            ps_ap,
                            wt[:, 2 * j:2 * j + 2, mcols],
                            xt[:, 2 * j:2 * j + 2, xcols],
                            start=(i == 0 and j == 0),
                            stop=(i == 2 and j == NDC // 2 - 1),
                            perf_mode=DR)

            def emit_q_proj(m):
                # Q projection chunk -> zero-padded per-head fp8 slots in QM
                ps = psp.tile([P, QB], f32, tag="ps", name="psq")
                proj_dr(ps[:], sb["w18"], sb["w1r"], sb["xq8"], sb["xqr"],
                        slice(m * P, (m + 1) * P), slice(None))
                for hp in range(2):
                    h = 2 * m + hp
                    base = hp * HD
                    nc.vector.tensor_scalar(
                        QM[base:base + HD,
                           (NKT + h) * QB:(NKT + h + 1) * QB],
                        ps[base:base + HD, :],
                        b1sb[base:base + HD, m:m + 1], None, Alu.add)

            def emit_k_proj(m, kbs=None):
                # K projection chunk -> fp8 kts slot m in KI.  The m=0
                # conversions run on the (otherwise idle) scalar engine so
                # the startup isn't serialized behind the DVE queue.
                for kb in (range(S // QB) if kbs is None else kbs):
                    ps = ps_a.tile([P, QB], f32, tag=f"pv{kb}",
                                   name="psk", bufs=1)
                    if kb == 0:
                        proj_dr(ps[:], sb["w28"], sb["w2r"],
                                sb["xk8a"], sb["xkra"],
                                slice(m * P, (m + 1) * P), slice(None))
                    else:
                        proj_dr(ps[:], sb["w28"], sb["w2r"],
                                sb["xk8b"], sb["xkrb"],
                                slice(m * P, (m + 1) * P),
                                slice((kb - 1) * QB, kb * QB))
                    if m == 0:
                        dst = KI0[kb][:, P:]
                        if kb == 0:
                            nc.scalar.add(dst, ps[:], b2sb[:, m:m + 1])
                        else:
                            nc.vector.tensor_scalar(
                                dst, ps[:], b2sb[:, m:m + 1], None, Alu.add)
                    elif m == 1:
                        dst = KI[m - 1][:, P + kb * QB: P + (kb + 1) * QB]
                        nc.vector.tensor_scalar(
                            dst, ps[:], b2sb[:, m:m + 1], None, Alu.add)
                    else:
                        dst = KI[m - 1][:, P + kb * QB: P + (kb + 1) * QB]
                        nc.scalar.add(dst, ps[:], b2sb[:, m:m + 1])

            VEv = VE[:].rearrange("p (t h c) -> p t h c", t=NKT, c=VW)

            def emit_v_proj(t0, t1):
                # V projection tiles -> fp16 VE (head-interleaved; the v bias
                # b3 is folded into the output-projection bias on the host,
                # so this is a pure convert, split across ACT and DVE)
                for t in range(t0, t1):
                    ps = ps_a.tile([P, D], f32, tag=f"pv{t % 4}",
                                   name="psv", bufs=1)
                    pairs = [(sb["xv8"], sb["w38"]), (sb["xvr"], sb["w38"]),
                             (sb["xv8"], sb["w3r"])]
                    for i, (xt, wt) in enumerate(pairs):
                        for j in range(NDC // 2):
                            nc.tensor.matmul(
                                ps[:],
                                xt[:, 2 * j:2 * j + 2, t * P:(t + 1) * P],
                                wt[:, 2 * j:2 * j + 2, :],
                                start=(i == 0 and j == 0),
                                stop=(i == 2 and j == NDC // 2 - 1),
                                perf_mode=DR)
                    src = ps[:].rearrange("p (h d) -> p h d", d=HD)
                    if t % 2 == 0:
                        nc.scalar.copy(VEv[:, t, :, 0:HD], src)
                    else:
                        nc.vector.tensor_copy(VEv[:, t, :, 0:HD], src)

            # ==== attention building blocks ====
            KIv = [KI[m][:].rearrange("p (s c) -> p s c", c=P)
                   for m in range(NDC - 1)]
            KI0v = [KI0[kb][:].rearrange("p (s c) -> p s c", c=P)
                    for kb in range(S // QB)]
            QMv = [QM[i][:].rearrange("p (s q) -> p s q", q=QB)
                   for i in range(2)]

            # exp engine per (head, group): A = exact exp on ACT; V =
            # Schraudolph int16 trick on DVE; P = two-stage lane for the
            # PSUM-blind Pool engine (ACT/DVE copies the f32 scores to an
            # f16 staging tile, Pool runs the Schraudolph from SBUF).  The
            # DVE/Pool share ramps up as projection/DMA duties wind down.
            EXP_ENG = {0: "AAVAAVAA", 1: "AVAAVAAV", 2: "AVAVAAVA",
                       4: "AVAVAVAA", 6: "AVAVAVAA"}
            EXP_ENG_LATE = "AVAVAVAV"

            def emit_scores_exp(h, g_lo=0, g_hi=NG, pes=None):
                m = h // 2
                engs = EXP_ENG.get(h, EXP_ENG_LATE)
                if pes is None:
                    pes = []
                for g in range(g_lo, g_hi):
                    sg = ps_s.tile([P, KTG * QB], f32, tag="s", name="s")
                    for j in range(KTG):
                        t = g * KTG + j
                        # slice 0: (identity, reversed-mask tile)
                        # slice 1: (kT tile, zero-padded per-head q)
                        if m == 0:
                            kv = KI0v[t // 4]
                            ks = 1 + t % 4
                        else:
                            kv = KIv[m - 1]
                            ks = 1 + t
                        ms = NKT - 1 - t
                        dq = NKT + h % 4 - ms
                        nc.tensor.matmul(
                            sg[:, j * QB:(j + 1) * QB],
                            kv[:, 0:ks + 1:ks, :],
                            QMv[h // 4][:, ms:ms + dq + 1:dq, :],
                            start=True, stop=True, perf_mode=DR)
                    if engs[g] == "A":
                        pe = pex.tile([P, KTG * QB], f16, tag="pe", name="pe")
                        nc.scalar.activation(pe[:], sg[:], ActF.Exp,
                                             scale=EXP_SCALE)
                        pes.append(pe[:])
                    elif engs[g] == "V":
                        pe = pex.tile([P, KTG * QB], i16, tag="pe", name="pe")
                        nc.vector.tensor_scalar(pe[:], sg[:], SCH_A, SCH_B,
                                                Alu.mult, op1=Alu.add)
                        pes.append(pe[:].bitcast(f16))
                    else:
                        st = psx.tile([P, KTG * QB], f16, tag="st", name="st")
                        if g % 2 == 0:
                            nc.scalar.copy(st[:], sg[:])
                        else:
                            nc.vector.tensor_copy(st[:], sg[:])
                        pe = pex.tile([P, KTG * QB], i16, tag="pe", name="pe")
                        nc.gpsimd.tensor_scalar(pe[:], st[:], SCH_A, SCH_B,
                                                Alu.mult, op1=Alu.add)
                        pes.append(pe[:].bitcast(f16))
                return pes

            def emit_pv(h, pes, t_outer=False):
                # p @ [V | ones]: p tiles stationary -> [q, 65] outputs
                if t_outer:
                    # last head: qb2/qb3 ride retired score-pool banks and
                    # pre-accumulate t<15; qb0/qb1 interleave by t; only
                    # four t=15 matmuls depend on the final exp
                    pvs = [ps_a.tile([P, VW], f32, tag=f"pv{qb}",
                                     name="pv", bufs=1) for qb in range(4)]
                    for qb in (2, 3):
                        for t in range(NKT - 1):
                            g, j = divmod(t, KTG)
                            nc.tensor.matmul(
                                pvs[qb][:],
                                pes[g][:, j * QB + qb * P:
                                       j * QB + (qb + 1) * P],
                                VEv[:, t, h, :],
                                start=(t == 0), stop=False)
                    for t in range(NKT):
                        g, j = divmod(t, KTG)
                        for qb in range(2):
                            nc.tensor.matmul(
                                pvs[qb][:],
                                pes[g][:, j * QB + qb * P:
                                       j * QB + (qb + 1) * P],
                                VEv[:, t, h, :],
                                start=(t == 0), stop=(t == NKT - 1))
                    t, (g, j) = NKT - 1, divmod(NKT - 1, KTG)
                    for qb in (2, 3):
                        nc.tensor.matmul(
                            pvs[qb][:],
                            pes[g][:, j * QB + qb * P:
                                   j * QB + (qb + 1) * P],
                            VEv[:, t, h, :],
                            start=False, stop=True)
                for qb in range(NQB):
                    if t_outer:
                        pv = pvs[qb]
                    else:
                        pv = ps_a.tile([P, VW], f32, tag=f"pv{qb}",
                                       name="pv", bufs=1)
                        for t in range(NKT):
                            g, j = divmod(t, KTG)
                            nc.tensor.matmul(
                                pv[:],
                                pes[g][:, j * QB + qb * P:
                                       j * QB + (qb + 1) * P],
                                VEv[:, t, h, :],
                                start=(t == 0), stop=(t == NKT - 1))
                    rcp = yp.tile([P, 1], f32, tag="rcp", name="rcp", bufs=2)
                    nc.vector.reciprocal(rcp[:], pv[:, HD:VW])
                    nc.vector.tensor_scalar(
                        out2[qb][:, h * HD:(h + 1) * HD],
                        pv[:, 0:HD], rcp[:], None, Alu.mult)

            oTT = pp.tile([P, NDC, QB], bf16, tag="oTT", name="oTT")

            def emit_transpose(m):
                # transpose out2[:, d-chunk m] -> oTT[:, m, :] ([d, q] layout)
                pst = ps_a.tile([P, QB], bf16, tag=f"pv{m}",
                                name="pst", bufs=1)
                for qt in range(NQB):
                    nc.tensor.matmul(
                        pst[:, qt * P:(qt + 1) * P],
                        out2[qt][:, m * P:(m + 1) * P],
                        sb["idb_d"][:],
                        start=True, stop=True, is_transpose=True)
                nc.vector.tensor_copy(oTT[:, m, :], pst[:])

            ypb = [pp.tile([P, D], f32, tag=f"ypb{qt}", name=f"ypb{qt}")
                   for qt in range(NQB)]

            def emit_tail_early(qt):
                # partial output projection over d-chunks 0..2 (+ bias)
                ps = ps_a.tile([P, D], f32, tag=f"pv{qt}",
                               name="psy", bufs=1)
                for m in range(NDC - 1):
                    nc.tensor.matmul(
                        ps[:], oTT[:, m, qt * P:(qt + 1) * P],
                        sb["wo_d"][:, m, :],
                        start=(m == 0), stop=(m == NDC - 2))
                nc.vector.tensor_tensor(ypb[qt][:], ps[:], bob[:],
                                        op=Alu.add)

            def emit_tail_late(qt):
                # last d-chunk + partial sum + store for one q block; the
                # y DMAs ride the SP and (idle by now) ACT hwdge queues
                ps = ps_a.tile([P, D], f32, tag=f"pv{qt}",
                               name="psy", bufs=1)
                nc.tensor.matmul(
                    ps[:], oTT[:, NDC - 1, qt * P:(qt + 1) * P],
                    sb["wo_d"][:, NDC - 1, :], start=True, stop=True)
                ysb = yp.tile([P, D], f32, tag="ysb", name="ysb", bufs=2)
                nc.vector.tensor_tensor(ysb[:], ps[:], ypb[qt][:],
                                        op=Alu.add)
                eng = nc.sync if qt % 2 == 0 else nc.scalar
                eng.dma_start(y[qt * P:(qt + 1) * P, :], ysb[:])

            # ==== PE p-state warm-up ====
            wps = ps_a.tile([P, D], f32, tag="pv0", name="wps", bufs=1)
            for w in range(15):
                nc.tensor.matmul(wps[:, 0:P], ones_r[:], ones_r[:, 0:P],
                                 start=True, stop=True)

            # ==== software-pipelined emission ====
            # PE is in-order: feed the activation engine (the critical
            # resource) as early and as continuously as possible.
            # Q projection pairs share wide psum tiles so the conversion
            # chain is not serialized behind the "ps" ring.
            for mp in [0]:
                psq = ps_s.tile([P, KTG * QB], f32, tag="s", name="psq")
                for mh in range(2):
                    m = 2 * mp + mh
                    proj_dr(psq[:, mh * QB:(mh + 1) * QB],
                            sb["w18"], sb["w1r"], sb["xq8"], sb["xqr"],
                            slice(m * P, (m + 1) * P), slice(None))
                for mh in range(2):
                    m = 2 * mp + mh
                    for hp in range(2):
                        h = 2 * m + hp
                        base = hp * HD
                        nc.vector.tensor_scalar(
                            QM[h // 4][base:base + HD,
                                       (NKT + h % 4) * QB:
                                       (NKT + h % 4 + 1) * QB],
                            psq[base:base + HD, mh * QB:(mh + 1) * QB],
                            b1sb[base:base + HD, m:m + 1], None, Alu.add)
            _mark(nc, 'K0sc0')
            sc = {}
            sc[0] = []
            for kb in range(S // QB):
                emit_k_proj(0, kbs=[kb])
                emit_scores_exp(0, 2 * kb, 2 * kb + 2, pes=sc[0])
            for mp in [1]:
                psq = ps_s.tile([P, KTG * QB], f32, tag="s", name="psq")
                for mh in range(2):
                    m = 2 * mp + mh
                    proj_dr(psq[:, mh * QB:(mh + 1) * QB],
                            sb["w18"], sb["w1r"], sb["xq8"], sb["xqr"],
                            slice(m * P, (m + 1) * P), slice(None))
                for mh in range(2):
                    m = 2 * mp + mh
                    for hp in range(2):
                        h = 2 * m + hp
                        base = hp * HD
                        nc.vector.tensor_scalar(
                            QM[h // 4][base:base + HD,
                                       (NKT + h % 4) * QB:
                                       (NKT + h % 4 + 1) * QB],
                            psq[base:base + HD, mh * QB:(mh + 1) * QB],
                            b1sb[base:base + HD, m:m + 1], None, Alu.add)
            _mark(nc, 'K1')
            emit_k_proj(1)
            _mark(nc, 'sc1')
            sc[1] = emit_scores_exp(1)
            _mark(nc, 'K2')
            emit_k_proj(2)
            _mark(nc, 'sc2')
            sc[2] = emit_scores_exp(2)
            # broadcast bo across partitions via a K=1 matmul (emitted here
            # so its late DMA never gates the Q/K projections at the head of
            # the in-order PE stream)
            psb2 = ps_a.tile([P, D], f32, tag="pv1", name="psb2", bufs=1)
            nc.tensor.matmul(psb2[:], ones_r[:], borow[:], start=True,
                             stop=True)
            nc.vector.tensor_copy(bob[:], psb2[:])
            _mark(nc, 'V0-8')
            emit_v_proj(0, 8)
            _mark(nc, 'K3')
            emit_k_proj(3)
            _mark(nc, 'sc3')
            sc[3] = emit_scores_exp(3)
            _mark(nc, 'V8-16')
            emit_v_proj(8, 16)
            _mark(nc, 'pv0')
            emit_pv(0, sc.pop(0))
            _mark(nc, 'pv1')
            emit_pv(1, sc.pop(1))
            _mark(nc, 'sc4')
            sc[4] = emit_scores_exp(4)
            _mark(nc, 'pv2')
            emit_pv(2, sc.pop(2))
            _mark(nc, 'sc5')
            sc[5] = emit_scores_exp(5)
            _mark(nc, 'pv3')
            emit_pv(3, sc.pop(3))
            _mark(nc, 'T0')
            emit_transpose(0)
            _mark(nc, 'sc6')
            sc[6] = emit_scores_exp(6)
            _mark(nc, 'pv4')
            emit_pv(4, sc.pop(4))
            _mark(nc, 'T1')
            emit_transpose(1)
            _mark(nc, 'sc7')
            sc[7] = emit_scores_exp(7)
            _mark(nc, 'pv5')
            emit_pv(5, sc.pop(5))
            _mark(nc, 'T2')
            emit_transpose(2)
            _mark(nc, 'tailE')
            for qt in range(NQB):
                emit_tail_early(qt)
            _mark(nc, 'pv6')
            emit_pv(6, sc.pop(6))
            _mark(nc, 'pv7')
            emit_pv(7, sc.pop(7), t_outer=True)
            _mark(nc, 'tailL')
            for qt in range(NQB):
                pst = ps_a.tile([P, P], bf16, tag=f"pv{qt}",
                                name="pst3", bufs=1)
                nc.tensor.matmul(
                    pst[:], out2[qt][:, (NDC - 1) * P:], sb["idb_d"][:],
                    start=True, stop=True, is_transpose=True)
                nc.vector.tensor_copy(
                    oTT[:, NDC - 1, qt * P:(qt + 1) * P], pst[:])
                emit_tail_late(qt)

    nc.finalize()
    return nc


_CACHE = {}


def _get_nc():
    if "nc" not in _CACHE:
        _CACHE["nc"] = _build_nc()
    return _CACHE["nc"]


F8NP = cdt.np(cdt.float8e4)
BF16NP = cdt.np(cdt.bfloat16)


def _to_chunked(a, inner):
    """[D, N] -> [P, NDC, N] with row d = c*P+p mapped to [p, c, :]."""
    return np.ascontiguousarray(
        a.reshape(NDC, P, inner).transpose(1, 0, 2))


def _split8(a):
    """Double-fp8 decomposition: a ~ hi + lo with both parts fp8e4."""
    hi = a.astype(F8NP)
    lo = (a - hi.astype(np.float32)).astype(F8NP)
    return hi, lo


def _prep_shared(W1, b1, W2, b2, W3, b3, Wo, bo):
    f = np.float32
    id8 = np.zeros((P, P), f)
    np.fill_diagonal(id8, IDENTV)
    idb = np.zeros((P, P), f)
    np.fill_diagonal(idb, 1.0)
    shared = {
        "id8_d": id8.astype(F8NP),
        "idb_d": idb.astype(BF16NP),
        "wo_d": _to_chunked(np.ascontiguousarray(np.asarray(Wo, f).T),
                            D).astype(BF16NP),
        "b1_d": np.ascontiguousarray(
            (np.asarray(b1, f) * f(WS)).reshape(NDC, P).T),
        "b2_d": np.ascontiguousarray(
            (np.asarray(b2, f) * f(WS)).reshape(NDC, P).T),
        # v-bias folds through the attention average and the output
        # projection exactly: y = (attn + b3) @ Wo.T + bo
        "bo_d": (np.asarray(bo, f)
                 + np.asarray(b3, f) @ np.asarray(Wo, f).T
                 ).reshape(1, D).copy(),
    }
    for nm, W in [("w1", W1), ("w2", W2), ("w3", W3)]:
        hi, lo = _split8(_to_chunked(np.asarray(W, f).T * f(WS), D))
        shared[nm + "8"] = hi
        shared[nm + "r"] = lo
    return shared


def build_in_maps(q_in, k_in, v_in, mask, W1, b1, W2, b2, W3, b3, Wo, bo):
    f = np.float32
    q_in = np.asarray(q_in, f)
    k_in = np.asarray(k_in, f)
    v_in = np.asarray(v_in, f)
    mask = np.asarray(mask)
    shared = _prep_shared(W1, b1, W2, b2, W3, b3, Wo, bo)
    kc = [_split8(_to_chunked(k_in[b].T, S)) for b in range(B)]
    kc = [(np.ascontiguousarray(h[:, :, :QB]),
           np.ascontiguousarray(h[:, :, QB:]),
           np.ascontiguousarray(l[:, :, :QB]),
           np.ascontiguousarray(l[:, :, QB:])) for h, l in kc]
    vc = [_split8(_to_chunked(v_in[b].T, S)) for b in range(B)]
    in_maps = []
    for c in range(NCORES):
        b, r = divmod(c, RPB)
        sl = slice(r * QB, (r + 1) * QB)
        # masked (m==0) entries get MASKV (=-120), unmasked get 0
        mt = (f(1.0) - mask[b, 0, sl, :].T.astype(f)) * f(MASKV)
        q8, qr = _split8(_to_chunked(
            np.ascontiguousarray(q_in[b, sl, :].T), QB))
        in_maps.append({
            "xq8": q8, "xqr": qr,
            "xk8a": kc[b][0], "xk8b": kc[b][1],
            "xkra": kc[b][2], "xkrb": kc[b][3],
            "xv8": vc[b][0], "xvr": vc[b][1],
            "m_d": np.ascontiguousarray(
                mt.reshape(NKT, P, QB)[::-1].transpose(1, 0, 2)).astype(F8NP),
            **shared,
        })
    return in_maps


def kernel(q_in, k_in, v_in, mask, W1, b1, W2, b2, W3, b3, Wo, bo):
    f = np.float32
    nc = _get_nc()
    in_maps = build_in_maps(q_in, k_in, v_in, mask, W1, b1, W2, b2, W3, b3,
                            Wo, bo)
    res = run_bass_kernel_spmd(nc, in_maps, list(range(NCORES)))
    out = np.empty((B, S, D), f)
    for c in range(NCORES):
        b, r = divmod(c, RPB)
        out[b, r * QB:(r + 1) * QB, :] = res.results[c]["y"]
    return out



# revision 38
# speedup vs baseline: 1.1908x; 1.0032x over previous
"""Multi-head attention on 8 Trainium2 NeuronCores — fp8 DoubleRow version.

Sharding: batch (2) x query-row-block (4) -> 8 cores; each core computes full
attention for its 512 query rows of one batch (K/V projected for all keys).

Techniques vs the fp32r baseline:
  - QKV projections run as fp8e4 DoubleRow matmuls on a host-side double-fp8
    decomposition (x ~ x8 + xr, W ~ w8 + wr, keeping the three first-order
    products) giving ~bf16 accuracy at fp8-DR speed.  Weights are scaled x8
    on host so fp8 quantization stays in e4m3 normal range.
  - Scores: one DoubleRow matmul per (head, key-tile) computes q.k AND adds
    the attention mask: slice 0 contracts the zero-padded per-head q against
    the two-head kT tile; slice 1 multiplies a 128*I identity into a {0,-120}
    mask tile.  PSUM gets qk - 15360*(1-m), i.e. s - 30*(1-m) after the 2^-9
    exp() activation scale (which also folds 1/sqrt(HD) and the x8 weight
    scales).  exp(s-30) underflows to exactly 0, matching the reference mask.
  - p@V runs with exp tiles (fp16) as the stationary operand: output lands
    as [q, 64v+1ones] per head with only 65 moving rows per instruction, and
    softmax denominators are normalized by a per-partition reciprocal via
    tensor_scalar.  A cheap PE transpose pass restores the [d, q] layout for
    the bf16 output projection.
Host side only reshapes/quantizes inputs and concatenates outputs.
"""

import numpy as np
import concourse.bass as bass
import concourse.mybir as mybir
from concourse import bacc
from concourse.dt import dt as cdt
from concourse.tile import TileContext
from concourse.bass_utils import run_bass_kernel_spmd

B, S, D, H, HD = 2, 2048, 512, 8, 64
P = 128
NCORES = 8
RPB = 4              # q-row blocks per batch
QB = S // RPB        # 512 query rows per core
NDC = D // P         # 4 chunks of the model dim
NKT = S // P         # 16 key tiles
KTG = 2              # key tiles per scores/exp group
NG = NKT // KTG      # 8 groups
VW = HD + 1          # 65 columns per head in VE (64 v + 1 ones)
NQB = QB // P        # 4 query-partition blocks

WS = 8.0             # host-side weight scale (keeps fp8 weights normal)
EXP_SCALE = 1.0 / 512.0   # 2^-9: undoes WS*WS and applies 1/sqrt(HD)
MASKV = -120.0       # mask tile value for masked-out entries
IDENTV = 128.0       # identity scale; IDENTV*MASKV*EXP_SCALE = -30
ONESV = 8.0          # V ones-column value (den = 8*sum(p); num = 8*sum(p*v))

# Schraudolph fp16 exp for the DVE/Pool engines: exp(psum/512) ~=
# bitcast_f16(int16(round(psum * 2*log2(e) + 15300))).  One tensor_scalar
# (mult, add) with an int16 output gives the rounded bits; the int16 tile is
# bitcast to f16 when consumed.  Max relative error ~3.3% on a sawtooth;
# applied to half the score tiles it costs ~1.5e-3 extra end-to-end error.
SCH_A = 2 * 1.4426950408889634
SCH_B = 15300.0

f32 = mybir.dt.float32
f32r = mybir.dt.float32r
bf16 = mybir.dt.bfloat16
f16 = mybir.dt.float16
f8 = mybir.dt.float8e4
u32 = mybir.dt.uint32
i16 = mybir.dt.int16
Alu = mybir.AluOpType
ActF = mybir.ActivationFunctionType
DR = mybir.MatmulPerfMode.DoubleRow

QM_W = (NKT + H // 2) * QB         # reversed mask tiles + 4 q slots
KI_W = P + S                       # fp8 identity + one d-chunk of kts

PHASES = []                        # (label, first_instruction_id) markers


def _mark(nc, label):
    PHASES.append((label, nc.get_next_instruction_name()))


def _build_nc():
    nc = bacc.Bacc("TRN2", target_bir_lowering=False, debug=False,
                   num_devices=NCORES)
    dram = {}
    for nm, shape, dt_ in [
        ("xq8", [P, NDC, QB], f8), ("xqr", [P, NDC, QB], f8),
        ("xk8a", [P, NDC, QB], f8), ("xk8b", [P, NDC, S - QB], f8),
        ("xkra", [P, NDC, QB], f8), ("xkrb", [P, NDC, S - QB], f8),
        ("xv8", [P, NDC, S], f8), ("xvr", [P, NDC, S], f8),
        ("m_d", [P, NKT, QB], f8),
        ("w18", [P, NDC, D], f8), ("w1r", [P, NDC, D], f8),
        ("w28", [P, NDC, D], f8), ("w2r", [P, NDC, D], f8),
        ("w38", [P, NDC, D], f8), ("w3r", [P, NDC, D], f8),
        ("wo_d", [P, NDC, D], bf16),
        ("id8_d", [P, P], f8), ("idb_d", [P, P], bf16),
        ("b1_d", [P, NDC], f32), ("b2_d", [P, NDC], f32),
        ("bo_d", [1, D], f32),
    ]:
        dram[nm] = nc.dram_tensor(nm, shape, dt_, kind="ExternalInput")
    y = nc.dram_tensor("y", [QB, D], f32, kind="ExternalOutput")

    with TileContext(nc) as tc, nc.allow_low_precision("fp8 attention"):
        with (
            tc.tile_pool(name="persist", bufs=1) as pp,
            tc.tile_pool(name="small", bufs=1) as sp,
            tc.tile_pool(name="ps_s", bufs=2, space="PSUM") as ps_s,
            tc.tile_pool(name="ps_a", bufs=4, space="PSUM") as ps_a,
            tc.tile_pool(name="pex", bufs=28) as pex,
            tc.tile_pool(name="yp", bufs=2) as yp,
        ):
            # ---- persistent SBUF tiles ----
            KI = [pp.tile([P, KI_W], f8, tag=f"KI{m}", name=f"KI{m}")
                  for m in range(1, NDC)]
            KI0 = [pp.tile([P, P + QB], f8, tag=f"KI0_{kb}",
                           name=f"KI0_{kb}") for kb in range(S // QB)]
            QM = [pp.tile([P, QM_W], f8, tag=f"QM{i}", name=f"QM{i}")
                  for i in range(2)]
            VE = pp.tile([P, NKT * H * VW], f16, tag="VE", name="VE")
            sb = {}
            for nm, w, dt_ in [
                ("xq8", NDC * QB, f8), ("xqr", NDC * QB, f8),
                ("xk8a", NDC * QB, f8), ("xk8b", NDC * (S - QB), f8),
                ("xkra", NDC * QB, f8), ("xkrb", NDC * (S - QB), f8),
                ("xv8", NDC * S, f8), ("xvr", NDC * S, f8),
                ("w18", NDC * D, f8), ("w1r", NDC * D, f8),
                ("w28", NDC * D, f8), ("w2r", NDC * D, f8),
                ("w38", NDC * D, f8), ("w3r", NDC * D, f8),
                ("wo_d", NDC * D, bf16), ("idb_d", P, bf16),
            ]:
                inner = w // (NDC if nm not in ("idb_d",) else 1)
                if nm == "idb_d":
                    sb[nm] = pp.tile([P, P], dt_, tag=nm, name=nm)
                else:
                    sb[nm] = pp.tile([P, NDC, inner], dt_, tag=nm, name=nm)
            out2 = [pp.tile([P, D], bf16, tag=f"o2_{qb}", name=f"o2_{qb}")
                    for qb in range(NQB)]
            outT = [pp.tile([P, QB], bf16, tag=f"oT{m}", name=f"oT{m}")
                    for m in range(NDC)]

            b1sb = sp.tile([P, NDC], f32, tag="b1sb", name="b1sb")
            b2sb = sp.tile([P, NDC], f32, tag="b2sb", name="b2sb")
            borow = sp.tile([1, D], f32r, tag="borow", name="borow")
            ones_r = sp.tile([1, P], f32r, tag="ones_r", name="ones_r")
            nc.vector._memset_packed(ones_r[:].bitcast(u32), 0x3F800000)

            # bulk DMAs split across SP and Pool rings; Q/K/mask inputs
            # first so the attention pipeline can start early
            for nm in ["xq8", "w18", "xqr"]:
                nc.sync.dma_start(sb[nm][:], dram[nm][:])
            nc.sync.dma_start(sb["xk8a"][:], dram["xk8a"][:])
            nc.sync.dma_start(sb["w28"][:], dram["w28"][:])
            nc.sync.dma_start(b1sb[:], dram["b1_d"][:])
            nc.sync.dma_start(sb["xk8b"][:], dram["xk8b"][:])
            for kb in range(S // QB):
                nc.sync.dma_start(KI0[kb][:, 0:P], dram["id8_d"][:])
            for m in range(NDC - 1):
                nc.sync.dma_start(KI[m][:, 0:P], dram["id8_d"][:])
            nc.sync.dma_start(sb["wo_d"][:], dram["wo_d"][:])
            nc.sync.dma_start(borow[:], dram["bo_d"][:].squeeze().bitcast(f32r))
            nc.sync.dma_start(sb["idb_d"][:], dram["idb_d"][:])
            nc.gpsimd.dma_start(sb["w1r"][:], dram["w1r"][:])
            nc.gpsimd.dma_start(sb["xkra"][:], dram["xkra"][:])
            nc.gpsimd.dma_start(sb["w2r"][:], dram["w2r"][:])
            nc.gpsimd.dma_start(
                QM[0][:, (NKT // 2) * QB:NKT * QB].rearrange(
                    "p (t q) -> p t q", t=NKT // 2),
                dram["m_d"][:, NKT // 2:, :])
            nc.gpsimd.dma_start(b2sb[:], dram["b2_d"][:])
            nc.gpsimd.dma_start(
                QM[0][:, 0:(NKT // 2) * QB].rearrange(
                    "p (t q) -> p t q", t=NKT // 2),
                dram["m_d"][:, 0:NKT // 2, :])
            nc.sync.dma_start(
                QM[1][:, 0:NKT * QB].rearrange(
                    "p (t q) -> p t q", t=NKT), dram["m_d"][:])
            for nm in ["xkrb", "xv8", "w38", "xvr", "w3r"]:
                nc.gpsimd.dma_start(sb[nm][:], dram[nm][:])

            # zero the per-head q slots (conversions fill 64 rows per slot)
            qmz = QM[0][:, NKT * QB:].bitcast(u32)
            nc.scalar.mul(qmz, qmz, 0.0)
            qmz1 = QM[1][:, NKT * QB:].bitcast(u32)
            nc.vector._memset_packed(qmz1, 0)
            # V ones-columns
            VEv = VE[:].rearrange("p (t h c) -> p t h c", t=NKT, c=VW)
            nc.gpsimd.memset(VEv[:, :, :, HD:VW], ONESV)

            def proj_dr(ps_ap, wmain, wres, xmain, xres, mcols, xcols):
                """psum += (w8+wr).T x8 + w8.T xr over 4 chunks, 3 DR mms."""
                pairs = [(wmain, xmain), (wmain, xres), (wres, xmain)]
                for i, (wt, xt) in enumerate(pairs):
                    for j in range(NDC // 2):
                        nc.tensor.matmul(
                            ps_ap,
                            wt[:, 2 * j:2 * j + 2, mcols],
                            xt[:, 2 * j:2 * j + 2, xcols],
                            start=(i == 0 and j == 0),
                            stop=(i == 2 and j == NDC // 2 - 1),
                            perf_mode=DR)

            def emit_q_proj(m):
                # Q projection chunk -> zero-padded per-head fp8 slots in QM
                ps = psp.tile([P, QB], f32, tag="ps", name="psq")
                proj_dr(ps[:], sb["w18"], sb["w1r"], sb["xq8"], sb["xqr"],
                        slice(m * P, (m + 1) * P), slice(None))
                for hp in range(2):
                    h = 2 * m + hp
                    base = hp * HD
                    nc.vector.tensor_scalar(
                        QM[base:base + HD,
                           (NKT + h) * QB:(NKT + h + 1) * QB],
                        ps[base:base + HD, :],
                        b1sb[base:base + HD, m:m + 1], None, Alu.add)

            def emit_k_proj(m, kbs=None):
                # K projection chunk -> fp8 kts slot m in KI.  The m=0
                # conversions run on the (otherwise idle) scalar engine so
                # the startup isn't serialized behind the DVE queue.
                for kb in (range(S // QB) if kbs is None else kbs):
                    ps = ps_a.tile([P, QB], f32, tag=f"pv{kb}",
                                   name="psk", bufs=1)
                    if kb == 0:
                        proj_dr(ps[:], sb["w28"], sb["w2r"],
                                sb["xk8a"], sb["xkra"],
                                slice(m * P, (m + 1) * P), slice(None))
                    else:
                        proj_dr(ps[:], sb["w28"], sb["w2r"],
                                sb["xk8b"], sb["xkrb"],
                                slice(m * P, (m + 1) * P),
                                slice((kb - 1) * QB, kb * QB))
                    if m == 0:
                        dst = KI0[kb][:, P:]
                        if kb == 0:
                            nc.scalar.add(dst, ps[:], b2sb[:, m:m + 1])
                        else:
                            nc.vector.tensor_scalar(
                                dst, ps[:], b2sb[:, m:m + 1], None, Alu.add)
                    elif m == 1:
                        dst = KI[m - 1][:, P + kb * QB: P + (kb + 1) * QB]
                        nc.vector.tensor_scalar(
                            dst, ps[:], b2sb[:, m:m + 1], None, Alu.add)
                    else:
                        dst = KI[m - 1][:, P + kb * QB: P + (kb + 1) * QB]
                        nc.scalar.add(dst, ps[:], b2sb[:, m:m + 1])

            VEv = VE[:].rearrange("p (t h c) -> p t h c", t=NKT, c=VW)

            def emit_v_proj(t0, t1):
                # V projection tiles -> fp16 VE (head-interleaved; the v bias
                # b3 is folded into the output-projection bias on the host,
                # so this is a pure convert, split across ACT and DVE)
                for t in range(t0, t1):
                    ps = ps_a.tile([P, D], f32, tag=f"pv{t % 4}",
                                   name="psv", bufs=1)
                    pairs = [(sb["xv8"], sb["w38"]), (sb["xvr"], sb["w38"]),
                             (sb["xv8"], sb["w3r"])]
                    for i, (xt, wt) in enumerate(pairs):
                        for j in range(NDC // 2):
                            nc.tensor.matmul(
                                ps[:],
                                xt[:, 2 * j:2 * j + 2, t * P:(t + 1) * P],
                                wt[:, 2 * j:2 * j + 2, :],
                                start=(i == 0 and j == 0),
                                stop=(i == 2 and j == NDC // 2 - 1),
                                perf_mode=DR)
                    src = ps[:].rearrange("p (h d) -> p h d", d=HD)
                    if t % 2 == 0:
                        nc.scalar.copy(VEv[:, t, :, 0:HD], src)
                    else:
                        nc.vector.tensor_copy(VEv[:, t, :, 0:HD], src)

            # ==== attention building blocks ====
            KIv = [KI[m][:].rearrange("p (s c) -> p s c", c=P)
                   for m in range(NDC - 1)]
            KI0v = [KI0[kb][:].rearrange("p (s c) -> p s c", c=P)
                    for kb in range(S // QB)]
            QMv = [QM[i][:].rearrange("p (s q) -> p s q", q=QB)
                   for i in range(2)]

            # exp engine per (head, group): A = exact exp on ACT; V =
            # Schraudolph int16 trick on DVE; P = two-stage lane for the
            # PSUM-blind Pool engine (ACT/DVE copies the f32 scores to an
            # f16 staging tile, Pool runs the Schraudolph from SBUF).  The
            # DVE/Pool share ramps up as projection/DMA duties wind down.
            EXP_ENG = {0: "AAVAAVAA", 1: "AVAAVAAV", 2: "AVAVAAVA",
                       4: "AVAVAVAA", 6: "AVAVAVAA"}
            EXP_ENG_LATE = "AVAVAVAV"

            def emit_scores_exp(h, g_lo=0, g_hi=NG, pes=None):
                m = h // 2
                engs = EXP_ENG.get(h, EXP_ENG_LATE)
                if pes is None:
                    pes = []
                for g in range(g_lo, g_hi):
                    sg = ps_s.tile([P, KTG * QB], f32, tag="s", name="s")
                    for j in range(KTG):
                        t = g * KTG + j
                        # slice 0: (identity, reversed-mask tile)
                        # slice 1: (kT tile, zero-padded per-head q)
                        if m == 0:
                            kv = KI0v[t // 4]
                            ks = 1 + t % 4
                        else:
                            kv = KIv[m - 1]
                            ks = 1 + t
                        ms = NKT - 1 - t
                        dq = NKT + h % 4 - ms
                        nc.tensor.matmul(
                            sg[:, j * QB:(j + 1) * QB],
                            kv[:, 0:ks + 1:ks, :],
                            QMv[h // 4][:, ms:ms + dq + 1:dq, :],
                            start=True, stop=True, perf_mode=DR)
                    if engs[g] == "A":
                        pe = pex.tile([P, KTG * QB], f16, tag="pe", name="pe")
                        nc.scalar.activation(pe[:], sg[:], ActF.Exp,
                                             scale=EXP_SCALE)
                        pes.append(pe[:])
                    elif engs[g] == "V":
                        pe = pex.tile([P, KTG * QB], i16, tag="pe", name="pe")
                        nc.vector.tensor_scalar(pe[:], sg[:], SCH_A, SCH_B,
                                                Alu.mult, op1=Alu.add)
                        pes.append(pe[:].bitcast(f16))
                    else:
                        st = psx.tile([P, KTG * QB], f16, tag="st", name="st")
                        if g % 2 == 0:
                            nc.scalar.copy(st[:], sg[:])
                        else:
                            nc.vector.tensor_copy(st[:], sg[:])
                        pe = pex.tile([P, KTG * QB], i16, tag="pe", name="pe")
                        nc.gpsimd.tensor_scalar(pe[:], st[:], SCH_A, SCH_B,
                                                Alu.mult, op1=Alu.add)
                        pes.append(pe[:].bitcast(f16))
                return pes

            def emit_pv(h, pes, t_outer=False):
                # p @ [V | ones]: p tiles stationary -> [q, 65] outputs
                if t_outer:
                    # last head: qb2/qb3 ride retired score-pool banks and
                    # pre-accumulate t<15; qb0/qb1 interleave by t; only
                    # four t=15 matmuls depend on the final exp
                    pvs = [ps_a.tile([P, VW], f32, tag=f"pv{qb}",
                                     name="pv", bufs=1) for qb in range(4)]
                    for qb in (2, 3):
                        for t in range(NKT - 1):
                            g, j = divmod(t, KTG)
                            nc.tensor.matmul(
                                pvs[qb][:],
                                pes[g][:, j * QB + qb * P:
                                       j * QB + (qb + 1) * P],
                                VEv[:, t, h, :],
                                start=(t == 0), stop=False)
                    for t in range(NKT):
                        g, j = divmod(t, KTG)
                        for qb in range(2):
                            nc.tensor.matmul(
                                pvs[qb][:],
                                pes[g][:, j * QB + qb * P:
                                       j * QB + (qb + 1) * P],
                                VEv[:, t, h, :],
                                start=(t == 0), stop=(t == NKT - 1))
                    t, (g, j) = NKT - 1, divmod(NKT - 1, KTG)
                    for qb in (2, 3):
                        nc.tensor.matmul(
                            pvs[qb][:],
                            pes[g][:, j * QB + qb * P:
                                   j * QB + (qb + 1) * P],
                            VEv[:, t, h, :],
                            start=False, stop=True)
                for qb in range(NQB):
                    if t_outer:
                        pv = pvs[qb]
                    else:
                        pv = ps_a.tile([P, VW], f32, tag=f"pv{qb}",
                                       name="pv", bufs=1)
                        for t in range(NKT):
                            g, j = divmod(t, KTG)
                            nc.tensor.matmul(
                                pv[:],
                                pes[g][:, j * QB + qb * P:
                                       j * QB + (qb + 1) * P],
                                VEv[:, t, h, :],
                                start=(t == 0), stop=(t == NKT - 1))
                    rcp = yp.tile([P, 1], f32, tag="rcp", name="rcp", bufs=2)
                    nc.vector.reciprocal(rcp[:], pv[:, HD:VW])
                    nc.vector.tensor_scalar(
                        out2[qb][:, h * HD:(h + 1) * HD],
                        pv[:, 0:HD], rcp[:], None, Alu.mult)

            oTT = pp.tile([P, NDC, QB], bf16, tag="oTT", name="oTT")

            def emit_transpose(m):
                # transpose out2[:, d-chunk m] -> oTT[:, m, :] ([d, q] layout)
                pst = ps_a.tile([P, QB], bf16, tag=f"pv{m}",
                                name="pst", bufs=1)
                for qt in range(NQB):
                    nc.tensor.matmul(
                        pst[:, qt * P:(qt + 1) * P],
                        out2[qt][:, m * P:(m + 1) * P],
                        sb["idb_d"][:],
                        start=True, stop=True, is_transpose=True)
                nc.vector.tensor_copy(oTT[:, m, :], pst[:])

            tailps = {}

            def emit_tail_early(qt):
                # bias + output-projection chunks 0..2 accumulate in a ps_s
                # half (scores are done with the ring by now); chunk 3 and
                # the drain happen in emit_tail_late after the last heads
                if qt % 2 == 0:
                    tailps[qt // 2] = ps_s.tile([P, KTG * QB], f32, tag="s",
                                                name="psy")
                ps = tailps[qt // 2][:, (qt % 2) * D:(qt % 2 + 1) * D]
                nc.tensor.matmul(ps, ones_r[:], borow[:], start=True,
                                 stop=False)
                for m in range(NDC - 1):
                    nc.tensor.matmul(
                        ps, oTT[:, m, qt * P:(qt + 1) * P],
                        sb["wo_d"][:, m, :], start=False, stop=False)

            def emit_tail_late(qt):
                # last d-chunk, then the (idle by now) ACT engine drains the
                # finished psum; y DMAs ride the SP and ACT hwdge queues
                ps = tailps[qt // 2][:, (qt % 2) * D:(qt % 2 + 1) * D]
                nc.tensor.matmul(
                    ps, oTT[:, NDC - 1, qt * P:(qt + 1) * P],
                    sb["wo_d"][:, NDC - 1, :], start=False, stop=True)
                ysb = yp.tile([P, D], f32, tag="ysb", name="ysb", bufs=2)
                nc.scalar.copy(ysb[:], ps)
                eng = nc.sync if qt % 2 == 0 else nc.scalar
                eng.dma_start(y[qt * P:(qt + 1) * P, :], ysb[:])

            # ==== PE p-state warm-up ====
            wps = ps_a.tile([P, D], f32, tag="pv0", name="wps", bufs=1)
            for w in range(15):
                nc.tensor.matmul(wps[:, 0:P], ones_r[:], ones_r[:, 0:P],
                                 start=True, stop=True)

            # ==== software-pipelined emission ====
            # PE is in-order: feed the activation engine (the critical
            # resource) as early and as continuously as possible.
            # Q projection pairs share wide psum tiles so the conversion
            # chain is not serialized behind the "ps" ring.
            for mp in [0]:
                psq = ps_s.tile([P, KTG * QB], f32, tag="s", name="psq")
                for mh in range(2):
                    m = 2 * mp + mh
                    proj_dr(psq[:, mh * QB:(mh + 1) * QB],
                            sb["w18"], sb["w1r"], sb["xq8"], sb["xqr"],
                            slice(m * P, (m + 1) * P), slice(None))
                for mh in range(2):
                    m = 2 * mp + mh
                    for hp in range(2):
                        h = 2 * m + hp
                        base = hp * HD
                        nc.vector.tensor_scalar(
                            QM[h // 4][base:base + HD,
                                       (NKT + h % 4) * QB:
                                       (NKT + h % 4 + 1) * QB],
                            psq[base:base + HD, mh * QB:(mh + 1) * QB],
                            b1sb[base:base + HD, m:m + 1], None, Alu.add)
            _mark(nc, 'K0sc0')
            sc = {}
            sc[0] = []
            for kb in range(S // QB):
                emit_k_proj(0, kbs=[kb])
                emit_scores_exp(0, 2 * kb, 2 * kb + 2, pes=sc[0])
            for mp in [1]:
                psq = ps_s.tile([P, KTG * QB], f32, tag="s", name="psq")
                for mh in range(2):
                    m = 2 * mp + mh
                    proj_dr(psq[:, mh * QB:(mh + 1) * QB],
                            sb["w18"], sb["w1r"], sb["xq8"], sb["xqr"],
                            slice(m * P, (m + 1) * P), slice(None))
                for mh in range(2):
                    m = 2 * mp + mh
                    for hp in range(2):
                        h = 2 * m + hp
                        base = hp * HD
                        nc.vector.tensor_scalar(
                            QM[h // 4][base:base + HD,
                                       (NKT + h % 4) * QB:
                                       (NKT + h % 4 + 1) * QB],
                            psq[base:base + HD, mh * QB:(mh + 1) * QB],
                            b1sb[base:base + HD, m:m + 1], None, Alu.add)
            _mark(nc, 'K1')
            emit_k_proj(1)
            _mark(nc, 'sc1')
            sc[1] = emit_scores_exp(1)
            _mark(nc, 'K2')
            emit_k_proj(2)
            _mark(nc, 'sc2')
            sc[2] = emit_scores_exp(2)

            _mark(nc, 'V0-8')
            emit_v_proj(0, 4)
            _mark(nc, 'K3')
            emit_k_proj(3)
            emit_v_proj(4, 8)
            _mark(nc, 'sc3')
            sc[3] = []
            emit_scores_exp(3, 0, 4, pes=sc[3])
            emit_v_proj(8, 12)
            emit_scores_exp(3, 4, NG, pes=sc[3])
            _mark(nc, 'V8-16')
            emit_v_proj(12, 16)
            _mark(nc, 'pv0')
            emit_pv(0, sc.pop(0))
            _mark(nc, 'pv1')
            emit_pv(1, sc.pop(1))
            _mark(nc, 'sc4')
            sc[4] = emit_scores_exp(4)
            _mark(nc, 'pv2')
            emit_pv(2, sc.pop(2))
            _mark(nc, 'sc5')
            sc[5] = emit_scores_exp(5)
            _mark(nc, 'pv3')
            emit_pv(3, sc.pop(3))
            _mark(nc, 'T0')
            emit_transpose(0)
            _mark(nc, 'sc6')
            sc[6] = emit_scores_exp(6)
            _mark(nc, 'pv4')
            emit_pv(4, sc.pop(4))
            _mark(nc, 'T1')
            emit_transpose(1)
            _mark(nc, 'sc7')
            sc[7] = emit_scores_exp(7)
            _mark(nc, 'pv5')
            emit_pv(5, sc.pop(5))
            _mark(nc, 'T2')
            emit_transpose(2)
            _mark(nc, 'tailE')
            for qt in range(NQB):
                emit_tail_early(qt)
            _mark(nc, 'pv6')
            emit_pv(6, sc.pop(6))
            _mark(nc, 'pv7')
            emit_pv(7, sc.pop(7), t_outer=True)
            _mark(nc, 'tailL')
            for qt in range(NQB):
                pst = ps_a.tile([P, P], bf16, tag=f"pv{qt}",
                                name="pst3", bufs=1)
                nc.tensor.matmul(
                    pst[:], out2[qt][:, (NDC - 1) * P:], sb["idb_d"][:],
                    start=True, stop=True, is_transpose=True)
                nc.vector.tensor_copy(
                    oTT[:, NDC - 1, qt * P:(qt + 1) * P], pst[:])
                emit_tail_late(qt)

    nc.finalize()
    return nc


_CACHE = {}


def _get_nc():
    if "nc" not in _CACHE:
        _CACHE["nc"] = _build_nc()
    return _CACHE["nc"]


F8NP = cdt.np(cdt.float8e4)
BF16NP = cdt.np(cdt.bfloat16)


def _to_chunked(a, inner):
    """[D, N] -> [P, NDC, N] with row d = c*P+p mapped to [p, c, :]."""
    return np.ascontiguousarray(
        a.reshape(NDC, P, inner).transpose(1, 0, 2))


def _split8(a):
    """Double-fp8 decomposition: a ~ hi + lo with both parts fp8e4."""
    hi = a.astype(F8NP)
    lo = (a - hi.astype(np.float32)).astype(F8NP)
    return hi, lo


def _prep_shared(W1, b1, W2, b2, W3, b3, Wo, bo):
    f = np.float32
    id8 = np.zeros((P, P), f)
    np.fill_diagonal(id8, IDENTV)
    idb = np.zeros((P, P), f)
    np.fill_diagonal(idb, 1.0)
    shared = {
        "id8_d": id8.astype(F8NP),
        "idb_d": idb.astype(BF16NP),
        "wo_d": _to_chunked(np.ascontiguousarray(np.asarray(Wo, f).T),
                            D).astype(BF16NP),
        "b1_d": np.ascontiguousarray(
            (np.asarray(b1, f) * f(WS)).reshape(NDC, P).T),
        "b2_d": np.ascontiguousarray(
            (np.asarray(b2, f) * f(WS)).reshape(NDC, P).T),
        # v-bias folds through the attention average and the output
        # projection exactly: y = (attn + b3) @ Wo.T + bo
        "bo_d": (np.asarray(bo, f)
                 + np.asarray(b3, f) @ np.asarray(Wo, f).T
                 ).reshape(1, D).copy(),
    }
    for nm, W in [("w1", W1), ("w2", W2), ("w3", W3)]:
        hi, lo = _split8(_to_chunked(np.asarray(W, f).T * f(WS), D))
        shared[nm + "8"] = hi
        shared[nm + "r"] = lo
    return shared


def build_in_maps(q_in, k_in, v_in, mask, W1, b1, W2, b2, W3, b3, Wo, bo):
    f = np.float32
    q_in = np.asarray(q_in, f)
    k_in = np.asarray(k_in, f)
    v_in = np.asarray(v_in, f)
    mask = np.asarray(mask)
    shared = _prep_shared(W1, b1, W2, b2, W3, b3, Wo, bo)
    kc = [_split8(_to_chunked(k_in[b].T, S)) for b in range(B)]
    kc = [(np.ascontiguousarray(h[:, :, :QB]),
           np.ascontiguousarray(h[:, :, QB:]),
           np.ascontiguousarray(l[:, :, :QB]),
           np.ascontiguousarray(l[:, :, QB:])) for h, l in kc]
    vc = [_split8(_to_chunked(v_in[b].T, S)) for b in range(B)]
    in_maps = []
    for c in range(NCORES):
        b, r = divmod(c, RPB)
        sl = slice(r * QB, (r + 1) * QB)
        # masked (m==0) entries get MASKV (=-120), unmasked get 0
        mt = (f(1.0) - mask[b, 0, sl, :].T.astype(f)) * f(MASKV)
        q8, qr = _split8(_to_chunked(
            np.ascontiguousarray(q_in[b, sl, :].T), QB))
        in_maps.append({
            "xq8": q8, "xqr": qr,
            "xk8a": kc[b][0], "xk8b": kc[b][1],
            "xkra": kc[b][2], "xkrb": kc[b][3],
            "xv8": vc[b][0], "xvr": vc[b][1],
            "m_d": np.ascontiguousarray(
                mt.reshape(NKT, P, QB)[::-1].transpose(1, 0, 2)).astype(F8NP),
            **shared,
        })
    return in_maps


def kernel(q_in, k_in, v_in, mask, W1, b1, W2, b2, W3, b3, Wo, bo):
    f = np.float32
    nc = _get_nc()
    in_maps = build_in_maps(q_in, k_in, v_in, mask, W1, b1, W2, b2, W3, b3,
                            Wo, bo)
    res = run_bass_kernel_spmd(nc, in_maps, list(range(NCORES)))
    out = np.empty((B, S, D), f)
    for c in range(NCORES):
        b, r = divmod(c, RPB)
        out[b, r * QB:(r + 1) * QB, :] = res.results[c]["y"]
    return out

